# revision 12
# baseline (speedup 1.0000x reference)
"""Trainium2 Bass kernel for BidirectionalAttentionalPromptEncoder.

Key algebraic fact: every batch element of the reference is IDENTICAL
(the input embeddings are broadcast over batch before any compute), so we
compute a single batch element on-device and broadcast on the host.

Distribution (8 NeuronCores, SPMD single program, data-driven per-core roles):
  core c:  seq = c&1 (0=prefix,1=suffix), stack = (c>>1)&1 (forward/backward
  LSTM stack), dir = (c>>2)&1 (direction inside the bidirectional layer).
  Each core runs one LSTM chain (seq,stack,dir) for layer 0 then layer 1,
  with AllGathers to exchange the per-direction hidden histories between
  layers.  The post-LSTM stages (gating, cross-attention, output transforms)
  are computed per-sequence; the host reads the prefix output from core 0
  and the suffix output from core 1.

All matmuls run in bf16 (f32 PSUM accumulation).  Time-reversal needed by
the bidirectional scans is handled by writing each step's hidden state at
both ascending and descending offsets (H_loc / H_rev) and selecting with
per-core 0/1 mask inputs, keeping the program SPMD-uniform.
"""

import math
import sys

sys.path.insert(0, "/opt/trn_rl_repo")

import ml_dtypes
import numpy as np

import concourse.bass as bass
import concourse.mybir as mybir
import concourse.tile as tile
from concourse.bass_utils import run_bass_kernel_spmd
from concourse.masks import make_identity

BF16 = mybir.dt.bfloat16
F32 = mybir.dt.float32
AF = mybir.ActivationFunctionType
ALU = mybir.AluOpType

C = 1024
T = 64            # prefix_length == suffix_length
H2 = 512
NH = 8
HD = C // NH      # 128
G = 4 * H2        # 2048 lstm gate dim
NC = 8            # cores
KC_IN = C // 128  # 8 input-dim chunks
KC_H = H2 // 128  # 4 hidden-dim chunks
MC_G = G // 128   # 16 gate-dim chunks
HW = KC_H * T     # 256 cols of one hidden history

_CACHE = {}


def _bf(x):
    return np.ascontiguousarray(np.asarray(x, np.float32).astype(ml_dtypes.bfloat16))


def _f32(x):
    return np.ascontiguousarray(np.asarray(x, np.float32))


def _sinusoid(t, c):
    pos = np.arange(t, dtype=np.float32)[:, None]
    div = np.exp((-math.log(10000.0) * np.arange(0, c, 2, dtype=np.float32) / c)
                 .astype(np.float32)).astype(np.float32)
    pe = np.zeros((t, c), np.float32)
    pe[:, 0::2] = np.sin(pos * div)
    pe[:, 1::2] = np.cos(pos * div)
    return pe


def _wtiles(w, kc, mc):
    """w [mc*128, kc*128] -> tiles [128, kc, mc, 128]: t[p,k,m,j] = w[m*128+j, k*128+p]."""
    nm, nk = w.shape
    assert nm == mc * 128 and nk == kc * 128
    out = w.reshape(mc, 128, kc, 128).transpose(3, 2, 0, 1)
    return np.ascontiguousarray(out)


def _wmoving(w, kc):
    """w [n, kc*128] -> [128, kc, n]: out[p,k,n] = w[n, k*128+p]."""
    n = w.shape[0]
    out = w.reshape(n, kc, 128).transpose(2, 1, 0)
    return np.ascontiguousarray(out)


def _pchunk(v):
    """v [m*128] -> [128, m] per-partition chunk layout."""
    m = v.shape[0] // 128
    return np.ascontiguousarray(v.reshape(m, 128).T)


def split_sync_waits(nc):
    """Walrus NO_STRUCT instructions hold limited sem-waits; split extras onto NoOps."""
    limited = (mybir.InstDrain, mybir.InstNoOp)
    fn = nc.m.functions[0]
    for blk in fn.blocks:
        newl = []
        for inst in blk.instructions:
            si = inst.sync_info
            maxw = 1
            if si is not None and len(si.on_wait) > maxw:
                waits = list(si.on_wait)
                pre, keep = waits[:-maxw], waits[-maxw:]
                for i, w in enumerate(pre):
                    nop = mybir.InstNoOp(name=f"{inst.name}-sw{i}", ins=[], outs=[])
                    nop.engine = inst.engine
                    nop.sync_info = mybir.SyncInfo(on_wait=[w], on_update=[])
                    newl.append(nop)
                si.on_wait = keep
                inst.sync_info = si
            newl.append(inst)
        blk.instructions = newl


def build_program():
    nc = bass.Bass()

    def din(name, shape, dt=BF16):
        return nc.dram_tensor(name, shape, dt, kind="ExternalInput")

    embT_d = din("embT", [128, KC_IN, T])
    wih_d = [din(f"wih{l}", [128, KC_IN, MC_G, 128]) for l in range(2)]
    whh_d = [din(f"whh{l}", [128, KC_H, MC_G, 128]) for l in range(2)]
    bias_d = [din(f"bias{l}", [128, MC_G], F32) for l in range(2)]
    selw_d = din("selw", [128, 16], F32)
    mseq_d = din("mseq", [128, 2], F32)
    gw1m_d = din("gw1m", [128, 2 * KC_IN, C])
    gw2s_d = din("gw2s", [128, KC_IN, MC_G, 128])
    gb1r_d = din("gb1r", [64, C])
    glgr_d = din("glgr", [64, C])
    glbr_d = din("glbr", [64, C])
    gb2c_d = din("gb2c", [128, MC_G], F32)
    wqs_d = din("wqs", [128, KC_IN, KC_IN, 128])
    wks_d = din("wks", [128, KC_IN, KC_IN, 128])
    bqc_d = din("bqc", [128, KC_IN], F32)
    bkc_d = din("bkc", [128, KC_IN], F32)
    wvm_d = din("wvm", [128, KC_IN, C])
    bvr_d = din("bvr", [64, C])
    wom_d = din("wom", [128, KC_IN, C])
    bor_d = din("bor", [64, C])
    ow1m_d = din("ow1m", [128, KC_IN, 2 * C])
    ob1r_d = din("ob1r", [64, 2 * C])
    og1r_d = din("og1r", [64, 2 * C])
    obe1r_d = din("obe1r", [64, 2 * C])
    ow2m_d = din("ow2m", [128, 2 * KC_IN, C])
    ob2r_d = din("ob2r", [64, C])
    og2r_d = din("og2r", [64, C])
    obe2r_d = din("obe2r", [64, C])
    lngr_d = din("lngr", [64, C])
    lnbr_d = din("lnbr", [64, C])
    po_d = nc.dram_tensor("po_out", [64, C], F32, kind="ExternalOutput")

    with tile.TileContext(nc) as tc:
        with (
            tc.tile_pool(name="w", bufs=1) as wp,          # persistent constants/state
            tc.tile_pool(name="wih", bufs=2) as wihp,      # streamed lstm input weights
            tc.tile_pool(name="whh", bufs=2) as whhp,      # streamed lstm recurrent weights
            tc.tile_pool(name="pw", bufs=2) as pwp,        # streamed post weights (16KB tiles)
            tc.tile_pool(name="rep", bufs=3) as repp,      # streamed replicated biases
            tc.tile_pool(name="sm", bufs=2) as sm,         # small working tiles
            tc.tile_pool(name="act", bufs=1) as ap,        # activations
            tc.tile_pool(name="ln", bufs=1) as lnp,        # LN scratch
            tc.tile_pool(name="ps", bufs=1, space="PSUM") as ps,
            tc.tile_pool(name="ps2", bufs=2, space="PSUM") as ps2,
            tc.tile_pool(name="dram", bufs=1, space="DRAM") as dram,
        ):
            # ---- constants / inputs to SBUF
            If32 = wp.tile([128, 128], F32)
            make_identity(nc, If32[:])
            Ibf = wp.tile([128, 128], BF16)
            nc.vector.tensor_copy(Ibf[:], If32[:])

            embT = wp.tile([128, KC_IN, T], BF16)
            nc.sync.dma_start(embT[:], embT_d[:])
            selw = wp.tile([128, 16], F32)
            nc.sync.dma_start(selw[:], selw_d[:])
            mseq = wp.tile([128, 2], F32)
            nc.sync.dma_start(mseq[:], mseq_d[:])
            bias = [wp.tile([128, MC_G], F32, tag=f"bias{l}", name=f"bias{l}") for l in range(2)]
            for l in range(2):
                nc.sync.dma_start(bias[l][:], bias_d[l][:])

            whh = [whhp.tile([128, KC_H, MC_G, 128], BF16, tag="whh", name=f"whhl{l}") for l in range(2)]
            for l in range(2):
                nc.sync.dma_start(whh[l][:], whh_d[l][:])

            # state carried across the whole LSTM section
            H = [[wp.tile([128, KC_H, T], BF16, tag=f"H{l}{b}", name=f"H{l}{b}") for b in range(2)]
                 for l in range(2)]  # H[layer][0]=loc, [1]=rev
            xT1 = wp.tile([128, KC_IN, T], BF16)
            ginT = wp.tile([128, 2 * KC_IN, T], BF16)

            ag1_in = dram.tile([128, 2 * HW], BF16, tag="agin")
            ag1_out = dram.tile([NC * 128, 2 * HW], BF16, tag="agout")
            ag2_in = dram.tile([128, 2 * HW], BF16, tag="agin2")
            ag2_out = dram.tile([NC * 128, 2 * HW], BF16, tag="agout2")
            ag3_in = dram.tile([128, 2 * HW], BF16, tag="agin3")
            ag3_out = dram.tile([NC * 128, 2 * HW], BF16, tag="agout3")

            def x_precompute(layer, rhs):
                """X = Wih @ x + biases -> [128, gate-chunk, T] f32 sbuf."""
                xsb = wp.tile([128, MC_G, T], F32, tag="X")
                for half in range(2):
                    wih_h = wihp.tile([128, KC_IN, MC_G // 2, 128], BF16, tag="wih")
                    nc.sync.dma_start(
                        wih_h[:], wih_d[layer][:, :, half * 8:(half + 1) * 8, :])
                    xps = ps.tile([128, 1024], F32, tag="mm")
                    for mc in range(MC_G // 2):
                        m = half * (MC_G // 2) + mc
                        for kc in range(KC_IN):
                            nc.tensor.matmul(
                                xps[:, mc * T:(mc + 1) * T],
                                wih_h[:, kc, mc, :],
                                rhs[:, kc, :],
                                start=(kc == 0), stop=(kc == KC_IN - 1))
                        nc.vector.tensor_scalar_add(
                            xsb[:, m, :],
                            xps[:, mc * T:(mc + 1) * T],
                            bias[layer][:, m:m + 1])
                return xsb

            def lstm_layer(layer, X):
                c_sb = sm.tile([128, KC_H], F32, tag="c_sb")
                h_sb = sm.tile([128, KC_H], BF16, tag="h_sb")
                hloc, hrev = H[layer]
                for i in range(T):
                    z = sm.tile([128, MC_G], F32, tag="z")
                    if i == 0:
                        nc.vector.tensor_copy(z[:], X[:, :, 0])
                    else:
                        zps = ps2.tile([128, MC_G], F32, tag="tp")
                        for mc in range(MC_G):
                            for kc in range(KC_H):
                                nc.tensor.matmul(
                                    zps[:, mc:mc + 1],
                                    whh[layer][:, kc, mc, :],
                                    h_sb[:, kc:kc + 1],
                                    start=(kc == 0), stop=(kc == KC_H - 1))
                        nc.vector.tensor_add(z[:], zps[:], X[:, :, i])
                    sif = sm.tile([128, 8], F32, tag="sif")
                    nc.scalar.activation(sif[:], z[:, 0:8], AF.Sigmoid)
                    tg = sm.tile([128, 4], F32, tag="tg")
                    nc.scalar.activation(tg[:], z[:, 8:12], AF.Tanh)
                    so = sm.tile([128, 4], F32, tag="so")
                    nc.scalar.activation(so[:], z[:, 12:16], AF.Sigmoid)
                    ig = sm.tile([128, 4], F32, tag="ig")
                    nc.vector.tensor_mul(ig[:], sif[:, 0:4], tg[:])
                    if i == 0:
                        nc.vector.tensor_copy(c_sb[:], ig[:])
                    else:
                        nc.vector.tensor_mul(c_sb[:], sif[:, 4:8], c_sb[:])
                        nc.vector.tensor_add(c_sb[:], c_sb[:], ig[:])
                    tc_ = sm.tile([128, 4], F32, tag="tc_")
                    nc.scalar.activation(tc_[:], c_sb[:], AF.Tanh)
                    nc.vector.tensor_mul(h_sb[:], so[:], tc_[:])
                    # write history at ascending and descending offsets
                    nc.vector.tensor_copy(hloc[:, :, i], h_sb[:])
                    nc.vector.tensor_copy(hrev[:, :, T - 1 - i], h_sb[:])

            def allgather(inp_flat, ag_in, ag_out):
                all_sb = wp.tile([128, NC, 2 * HW], BF16, tag="allg")
                nc.sync.dma_start(ag_in[:], inp_flat)
                nc.gpsimd.collective_compute(
                    "AllGather", ALU.bypass,
                    ins=[ag_in.opt()], outs=[ag_out.opt()],
                    replica_groups=[list(range(NC))])
                for r in range(NC):
                    nc.sync.dma_start(all_sb[:, r, :], ag_out[r * 128:(r + 1) * 128, :])
                return all_sb

            # ================= LSTM =================
            X0 = x_precompute(0, embT)
            lstm_layer(0, X0)

            hcat0 = sm.tile([128, 2 * HW], BF16, tag="hcat")
            nc.vector.tensor_copy(hcat0[:, 0:HW], H[0][0][:].rearrange('p a b -> p (a b)'))
            nc.vector.tensor_copy(hcat0[:, HW:], H[0][1][:].rearrange('p a b -> p (a b)'))
            all1 = allgather(hcat0[:], ag1_in, ag1_out)

            # xT1 group A (chunks 0-3) = dir0-core's history, group B = dir1-core's
            xf = xT1[:].rearrange('p a b -> p (a b)')
            for g in range(2):
                dst = xf[:, g * HW:(g + 1) * HW]
                first = True
                for k in range(4):
                    for L in range(2):
                        w_ap = selw[:, g * 8 + k * 2 + L: g * 8 + k * 2 + L + 1]
                        src = all1[:, k + 4 * g, L * HW:(L + 1) * HW]
                        if first:
                            nc.vector.tensor_scalar_mul(dst, src, w_ap)
                            first = False
                        else:
                            nc.vector.scalar_tensor_tensor(
                                out=dst, in0=src, scalar=w_ap, in1=dst,
                                op0=ALU.mult, op1=ALU.add)

            X1 = x_precompute(1, xT1)
            lstm_layer(1, X1)

            hcat1 = sm.tile([128, 2 * HW], BF16, tag="hcat")
            nc.vector.tensor_copy(hcat1[:, 0:HW], H[1][0][:].rearrange('p a b -> p (a b)'))
            nc.vector.tensor_copy(hcat1[:, HW:], H[1][1][:].rearrange('p a b -> p (a b)'))
            all2 = allgather(hcat1[:], ag2_in, ag2_out)

            # ginT: 4 chain groups x 256 cols; chain (stack,dir) sits at rank
            # seq + 2*stack + 4*dir; take loc if stack==dir else rev.
            mp = mseq[:, 0:1]
            ms = mseq[:, 1:2]
            gin_flat = ginT[:].rearrange('p a b -> p (a b)')
            for gi, (st, dr) in enumerate([(0, 0), (0, 1), (1, 0), (1, 1)]):
                L = 0 if st == dr else 1
                base = 2 * st + 4 * dr
                dst = gin_flat[:, gi * HW:(gi + 1) * HW]
                nc.vector.tensor_scalar_mul(dst, all2[:, base, L * HW:(L + 1) * HW], mp)
                nc.vector.scalar_tensor_tensor(
                    out=dst, in0=all2[:, base + 1, L * HW:(L + 1) * HW], scalar=ms,
                    in1=dst, op0=ALU.mult, op1=ALU.add)

            # ---- helpers for post stages
            def ln_(x, gam, bet, out, F):
                s = sm.tile([64, 1], F32, tag="ln_s")
                nc.vector.reduce_sum(s[:], x[:, :F], axis=mybir.AxisListType.X)
                negmu = sm.tile([64, 1], F32, tag="ln_negmu")
                nc.vector.tensor_scalar_mul(negmu[:], s[:], -1.0 / F)
                xc = lnp.tile([64, 2 * C], F32, tag="ln_xc")
                nc.vector.tensor_scalar_add(xc[:, :F], x[:, :F], negmu[:])
                sq = lnp.tile([64, 2 * C], F32, tag="ln_sq")
                ssum = sm.tile([64, 1], F32, tag="ln_ssum")
                nc.vector.scalar_tensor_tensor(
                    out=sq[:, :F], in0=xc[:, :F], scalar=1.0, in1=xc[:, :F],
                    op0=ALU.mult, op1=ALU.mult, accum_out=ssum[:])
                var = sm.tile([64, 1], F32, tag="ln_var")
                nc.vector.tensor_scalar_mul(var[:], ssum[:], 1.0 / F)
                sd = sm.tile([64, 1], F32, tag="ln_sd")
                nc.scalar.activation(sd[:], var[:], AF.Sqrt, bias=eps_t[:])
                rstd = sm.tile([64, 1], F32, tag="ln_rstd")
                nc.vector.reciprocal(rstd[:], sd[:])
                nc.vector.tensor_scalar_mul(xc[:, :F], xc[:, :F], rstd[:])
                nc.vector.tensor_mul(xc[:, :F], xc[:, :F], gam[:, :F])
                nc.vector.tensor_add(out[:, :F], xc[:, :F], bet[:, :F])

            def rep_tile(d, F):
                t = repp.tile([64, 2 * C], BF16, tag="rep")
                nc.sync.dma_start(t[:, :F], d[:])
                return t

            def transpose_to(dstT, src_nat, nchunks, ident, tagsuffix=""):
                """src_nat [64, nchunks*128] -> dstT [128, nchunks, T] bf16."""
                for kc in range(nchunks):
                    tp = ps2.tile([128, 64], F32, tag="tp")
                    nc.tensor.transpose(tp[:], src_nat[:, kc * 128:(kc + 1) * 128],
                                        ident[:64, :64])
                    nc.vector.tensor_copy(dstT[:, kc, :], tp[:])

            eps_t = wp.tile([64, 1], F32)
            nc.gpsimd.memset(eps_t[:], 1e-5)
            natA = ap.tile([64, 2 * C], F32, tag="natA")
            natB = ap.tile([64, 2 * C], F32, tag="natB")

            # ================= gate stage =================
            gps = ps.tile([64, 2 * C], F32, tag="mm")
            for ns in range(2):
                gw1_h = pwp.tile([128, 2 * KC_IN, 512], BF16, tag="big")
                nc.sync.dma_start(gw1_h[:], gw1m_d[:, :, ns * 512:(ns + 1) * 512])
                for kc in range(2 * KC_IN):
                    nc.tensor.matmul(
                        gps[:, ns * 512:(ns + 1) * 512],
                        ginT[:, kc, :],
                        gw1_h[:, kc, :],
                        start=(kc == 0), stop=(kc == 2 * KC_IN - 1))
            gb1r = rep_tile(gb1r_d, C)
            nc.vector.tensor_add(natA[:, :C], gps[:, :C], gb1r[:, :C])
            glgr = rep_tile(glgr_d, C)
            glbr = rep_tile(glbr_d, C)
            ln_(natA, glgr, glbr, natB, C)
            nc.scalar.activation(natA[:, :C], natB[:, :C], AF.Gelu)

            g_hT = ap.tile([128, KC_IN, T], BF16, tag="g_hT")
            transpose_to(g_hT, natA, KC_IN, If32)

            gb2c = wp.tile([128, MC_G], F32)
            nc.sync.dma_start(gb2c[:], gb2c_d[:])
            gatesT = ap.tile([128, MC_G, T], BF16, tag="gatesT")
            for half in range(2):
                gw2_h = pwp.tile([128, KC_IN, MC_G // 2, 128], BF16, tag="big")
                nc.sync.dma_start(gw2_h[:], gw2s_d[:, :, half * 8:(half + 1) * 8, :])
                gtv = ps.tile([128, 1024], F32, tag="mm")
                for mc in range(MC_G // 2):
                    m = half * (MC_G // 2) + mc
                    for kc in range(KC_IN):
                        nc.tensor.matmul(
                            gtv[:, mc * T:(mc + 1) * T],
                            gw2_h[:, kc, mc, :],
                            g_hT[:, kc, :],
                            start=(kc == 0), stop=(kc == KC_IN - 1))
                    nc.scalar.activation(
                        gatesT[:, m, :], gtv[:, mc * T:(mc + 1) * T],
                        AF.Sigmoid, bias=gb2c[:, m:m + 1])

            biT = ap.tile([128, KC_IN, T], BF16, tag="biT")
            gflat = gatesT[:].rearrange('p a b -> p (a b)')
            ginf = ginT[:].rearrange('p a b -> p (a b)')
            bif = biT[:].rearrange('p a b -> p (a b)')
            t1 = sm.tile([128, KC_IN * T], BF16, tag="bi_t1")
            nc.vector.tensor_mul(t1[:], gflat[:, 0:512], ginf[:, 0:512])
            nc.vector.tensor_mul(bif[:], gflat[:, 512:1024], ginf[:, 512:1024])
            nc.vector.tensor_add(bif[:], bif[:], t1[:])

            # exchange biT between sequences (pad AG buffer reuse shape)
            all3 = allgather(bif[:], ag3_in, ag3_out)
            kvT = ap.tile([128, KC_IN, T], BF16, tag="kvT")
            kvf = kvT[:].rearrange('p a b -> p (a b)')
            nc.vector.tensor_scalar_mul(kvf[:], all3[:, 1, 0:512], mp)
            nc.vector.scalar_tensor_tensor(
                out=kvf[:], in0=all3[:, 0, 0:512], scalar=ms, in1=kvf[:],
                op0=ALU.mult, op1=ALU.add)

            # ================= attention =================
            bqc = wp.tile([128, KC_IN], F32)
            nc.sync.dma_start(bqc[:], bqc_d[:])
            bkc = wp.tile([128, KC_IN], F32)
            nc.sync.dma_start(bkc[:], bkc_d[:])

            qT = ap.tile([128, KC_IN, T], BF16, tag="qT")
            kT = ap.tile([128, KC_IN, T], BF16, tag="kT")
            for (dst, wsd, bc, scale, rhs) in (
                (qT, wqs_d, bqc, 1.0 / math.sqrt(HD), biT),
                (kT, wks_d, bkc, 1.0, kvT),
            ):
                ws = pwp.tile([128, KC_IN, KC_IN, 128], BF16, tag="big")
                nc.sync.dma_start(ws[:], wsd[:])
                qv = ps.tile([128, 1024], F32, tag="mm")
                for mc in range(KC_IN):
                    for kc in range(KC_IN):
                        nc.tensor.matmul(
                            qv[:, mc * T:(mc + 1) * T],
                            ws[:, kc, mc, :],
                            rhs[:, kc, :],
                            start=(kc == 0), stop=(kc == KC_IN - 1))
                    nc.vector.tensor_scalar(
                        out=dst[:, mc, :], in0=qv[:, mc * T:(mc + 1) * T],
                        scalar1=bc[:, mc:mc + 1], scalar2=scale,
                        op0=ALU.add, op1=ALU.mult)

            wvm = pwp.tile([128, KC_IN, C], BF16, tag="big")
            nc.sync.dma_start(wvm[:], wvm_d[:])
            vps = ps.tile([64, 2 * C], F32, tag="mm")
            for ns in range(2):
                for kc in range(KC_IN):
                    nc.tensor.matmul(
                        vps[:, ns * 512:(ns + 1) * 512],
                        kvT[:, kc, :],
                        wvm[:, kc, ns * 512:(ns + 1) * 512],
                        start=(kc == 0), stop=(kc == KC_IN - 1))
            bvr = rep_tile(bvr_d, C)
            v_sb = ap.tile([64, C], BF16, tag="v_sb")
            nc.vector.tensor_add(v_sb[:], vps[:, :C], bvr[:, :C])

            sps = ps2.tile([64, NH, T], F32, tag="tp")
            for h in range(NH):
                nc.tensor.matmul(sps[:, h, :], qT[:, h, :], kT[:, h, :],
                                 start=True, stop=True)
            negmax = sm.tile([64, NH], F32, tag="negmax")
            nc.vector.tensor_reduce(negmax[:], sps[:], axis=mybir.AxisListType.X,
                                    op=ALU.max, negate=True)
            attn = ap.tile([64, NH, T], F32, tag="attn")
            sumexp = sm.tile([64, NH], F32, tag="sumexp")
            for h in range(NH):
                nc.scalar.activation(attn[:, h, :], sps[:, h, :], AF.Exp,
                                     bias=negmax[:, h:h + 1],
                                     accum_out=sumexp[:, h:h + 1])
            recip = sm.tile([64, NH], F32, tag="recip")
            nc.vector.reciprocal(recip[:], sumexp[:])

            attnT = ap.tile([64, NH, T], BF16, tag="attnT")
            for h in range(NH):
                tpa = ps2.tile([64, 64], F32, tag="tp")
                nc.tensor.transpose(tpa[:], attn[:, h, :], If32[:64, :64])
                nc.vector.tensor_copy(attnT[:, h, :], tpa[:])

            aops = ps.tile([64, 2 * C], F32, tag="mm")
            for h in range(NH):
                nc.tensor.matmul(aops[:, h * HD:(h + 1) * HD],
                                 attnT[:, h, :], v_sb[:, h * HD:(h + 1) * HD],
                                 start=True, stop=True)
            ao = ap.tile([64, C], BF16, tag="ao")
            for h in range(NH):
                nc.vector.tensor_scalar_mul(
                    ao[:, h * HD:(h + 1) * HD], aops[:, h * HD:(h + 1) * HD],
                    recip[:, h:h + 1])
            aoT = ap.tile([128, KC_IN, T], BF16, tag="aoT")
            for kc in range(KC_IN):
                tpb = ps2.tile([128, 64], BF16, tag="tpb")
                nc.tensor.transpose(tpb[:], ao[:, kc * 128:(kc + 1) * 128], Ibf[:64, :64])
                nc.vector.tensor_copy(aoT[:, kc, :], tpb[:])

            wom = pwp.tile([128, KC_IN, C], BF16, tag="big")
            nc.sync.dma_start(wom[:], wom_d[:])
            paps = ps.tile([64, 2 * C], F32, tag="mm")
            for ns in range(2):
                for kc in range(KC_IN):
                    nc.tensor.matmul(
                        paps[:, ns * 512:(ns + 1) * 512],
                        aoT[:, kc, :],
                        wom[:, kc, ns * 512:(ns + 1) * 512],
                        start=(kc == 0), stop=(kc == KC_IN - 1))

            # residual: bi natural + pa + bo
            nc.vector.tensor_copy(natA[:, :C], paps[:, :C])
            for kc in range(KC_IN):
                tpr = ps2.tile([64, 128], BF16, tag="tpb")
                nc.tensor.transpose(tpr[:], biT[:, kc, :], Ibf[:, :])
                nc.vector.tensor_add(natA[:, kc * 128:(kc + 1) * 128],
                                     natA[:, kc * 128:(kc + 1) * 128], tpr[:])
            bor = rep_tile(bor_d, C)
            nc.vector.tensor_add(natA[:, :C], natA[:, :C], bor[:, :C])

            lngr = rep_tile(lngr_d, C)
            lnbr = rep_tile(lnbr_d, C)
            ln_(natA, lngr, lnbr, natB, C)   # natB = po_pre

            # ================= output transform =================
            ppT = ap.tile([128, KC_IN, T], BF16, tag="ppT")
            transpose_to(ppT, natB, KC_IN, If32)

            h1ps = ps.tile([64, 2 * C], F32, tag="mm")
            for half in range(2):
                ow1_h = pwp.tile([128, KC_IN, C], BF16, tag="big")
                nc.sync.dma_start(ow1_h[:], ow1m_d[:, :, half * C:(half + 1) * C])
                for ns in range(2):
                    for kc in range(KC_IN):
                        nc.tensor.matmul(
                            h1ps[:, half * C + ns * 512: half * C + (ns + 1) * 512],
                            ppT[:, kc, :],
                            ow1_h[:, kc, ns * 512:(ns + 1) * 512],
                            start=(kc == 0), stop=(kc == KC_IN - 1))
            ob1r = rep_tile(ob1r_d, 2 * C)
            nc.vector.tensor_add(natA[:], h1ps[:], ob1r[:])
            og1r = rep_tile(og1r_d, 2 * C)
            obe1r = rep_tile(obe1r_d, 2 * C)
            ln_(natA, og1r, obe1r, natB, 2 * C)
            nc.scalar.activation(natA[:], natB[:], AF.Gelu)

            h1T = ap.tile([128, 2 * KC_IN, T], BF16, tag="h1T")
            transpose_to(h1T, natA, 2 * KC_IN, If32)

            pops = ps.tile([64, 2 * C], F32, tag="mm")
            for half in range(2):
                ow2_h = pwp.tile([128, 2 * KC_IN, 512], BF16, tag="big")
                nc.sync.dma_start(ow2_h[:], ow2m_d[:, :, half * 512:(half + 1) * 512])
                for kc in range(2 * KC_IN):
                    nc.tensor.matmul(
                        pops[:, half * 512:(half + 1) * 512],
                        h1T[:, kc, :],
                        ow2_h[:, kc, :],
                        start=(kc == 0), stop=(kc == 2 * KC_IN - 1))
            ob2r = rep_tile(ob2r_d, C)
            nc.vector.tensor_add(natA[:, :C], pops[:, :C], ob2r[:, :C])
            og2r = rep_tile(og2r_d, C)
            obe2r = rep_tile(obe2r_d, C)
            ln_(natA, og2r, obe2r, natB, C)
            nc.sync.dma_start(po_d[:], natB[:, :C])

    split_sync_waits(nc)
    return nc


def _prep_core_inputs(c, emb_pe, lstm_Wih, lstm_Whh, lstm_bih, lstm_bhh,
                      attn_w, attn_b, gate_w1, gate_b1, gate_lg, gate_lb,
                      gate_w2, gate_b2, out_w1, out_b1, out_g1, out_be1,
                      out_w2, out_b2, out_g2, out_be2, ln_g, ln_b):
    seq = c & 1
    stack = (c >> 1) & 1
    dr = (c >> 2) & 1
    asc = (stack == dr)

    x = emb_pe[seq]
    order = np.arange(T) if asc else np.arange(T)[::-1]
    xl = x[order]  # local time
    embT = _bf(xl.reshape(T, KC_IN, 128).transpose(2, 1, 0))  # [128, kc, t]

    m = {"embT": embT}
    for l in range(2):
        m[f"wih{l}"] = _bf(_wtiles(np.asarray(lstm_Wih[stack, l, dr]), KC_IN, MC_G))
        m[f"whh{l}"] = _bf(_wtiles(np.asarray(lstm_Whh[stack, l, dr]), KC_H, MC_G))
        m[f"bias{l}"] = _f32(_pchunk(np.asarray(lstm_bih[stack, l, dr])
                                     + np.asarray(lstm_bhh[stack, l, dr])))

    selw = np.zeros((16,), np.float32)
    k = seq + 2 * stack
    selw[0 * 8 + k * 2 + dr] = 1.0            # group A: dir0-core, L = mydir
    selw[1 * 8 + k * 2 + (1 - dr)] = 1.0      # group B: dir1-core, L = 1-mydir
    m["selw"] = _f32(np.broadcast_to(selw, (128, 16)))
    mseq = np.zeros((2,), np.float32)
    mseq[seq] = 1.0
    m["mseq"] = _f32(np.broadcast_to(mseq, (128, 2)))

    q = seq
    rep = lambda v: _bf(np.broadcast_to(np.asarray(v, np.float32).reshape(1, -1),
                                        (64, np.asarray(v).shape[-1])))
    m["gw1m"] = _bf(_wmoving(np.asarray(gate_w1[q]), 2 * KC_IN))
    m["gb1r"] = rep(gate_b1[q])
    m["glgr"] = rep(gate_lg[q])
    m["glbr"] = rep(gate_lb[q])
    m["gw2s"] = _bf(_wtiles(np.asarray(gate_w2[q]), KC_IN, MC_G))
    m["gb2c"] = _f32(_pchunk(np.asarray(gate_b2[q])))
    m["wqs"] = _bf(_wtiles(np.asarray(attn_w[q, 0]), KC_IN, KC_IN))
    m["wks"] = _bf(_wtiles(np.asarray(attn_w[q, 1]), KC_IN, KC_IN))
    m["bqc"] = _f32(_pchunk(np.asarray(attn_b[q, 0])))
    m["bkc"] = _f32(_pchunk(np.asarray(attn_b[q, 1])))
    m["wvm"] = _bf(_wmoving(np.asarray(attn_w[q, 2]), KC_IN))
    m["bvr"] = rep(attn_b[q, 2])
    m["wom"] = _bf(_wmoving(np.asarray(attn_w[q, 3]), KC_IN))
    m["bor"] = rep(attn_b[q, 3])
    m["ow1m"] = _bf(_wmoving(np.asarray(out_w1[q]), KC_IN))
    m["ob1r"] = rep(out_b1[q])
    m["og1r"] = rep(out_g1[q])
    m["obe1r"] = rep(out_be1[q])
    m["ow2m"] = _bf(_wmoving(np.asarray(out_w2[q]), 2 * KC_IN))
    m["ob2r"] = rep(out_b2[q])
    m["og2r"] = rep(out_g2[q])
    m["obe2r"] = rep(out_be2[q])
    m["lngr"] = rep(ln_g)
    m["lnbr"] = rep(ln_b)
    return m


def kernel(prefix_emb, suffix_emb, lstm_Wih, lstm_Whh, lstm_bih, lstm_bhh,
           attn_w, attn_b, gate_w1, gate_b1, gate_lg, gate_lb, gate_w2, gate_b2,
           out_w1, out_b1, out_g1, out_be1, out_w2, out_b2, out_g2, out_be2,
           ln_g, ln_b, batch_size):
    if "nc" not in _CACHE:
        _CACHE["nc"] = build_program()
    nc = _CACHE["nc"]

    pe = _sinusoid(T, C)
    emb_pe = (np.asarray(prefix_emb, np.float32) + pe,
              np.asarray(suffix_emb, np.float32) + pe)

    in_maps = [
        _prep_core_inputs(c, emb_pe, lstm_Wih, lstm_Whh, lstm_bih, lstm_bhh,
                          attn_w, attn_b, gate_w1, gate_b1, gate_lg, gate_lb,
                          gate_w2, gate_b2, out_w1, out_b1, out_g1, out_be1,
                          out_w2, out_b2, out_g2, out_be2,
                          np.asarray(ln_g, np.float32),
                          np.asarray(ln_b, np.float32))
        for c in range(NC)
    ]
    res = run_bass_kernel_spmd(nc, in_maps, core_ids=list(range(NC)))
    po = np.asarray(res.results[0]["po_out"], np.float32)
    so = np.asarray(res.results[1]["po_out"], np.float32)
    b = int(batch_size)
    po_b = np.broadcast_to(po[None], (b, T, C)).copy()
    so_b = np.broadcast_to(so[None], (b, T, C)).copy()
    return po_b, so_b


# revision 15
# speedup vs baseline: 77.6980x; 77.6980x over previous
"""Trainium2 Bass kernel for BidirectionalAttentionalPromptEncoder.

Key algebraic fact: every batch element of the reference is IDENTICAL
(the input embeddings are broadcast over batch before any compute), so we
compute a single batch element on-device and broadcast on the host.

Distribution (8 NeuronCores, SPMD single program, data-driven per-core roles):
  core c:  seq = c&1 (0=prefix,1=suffix), stack = (c>>1)&1 (forward/backward
  LSTM stack), dir = (c>>2)&1 (direction inside the bidirectional layer).
  Each core runs one LSTM chain (seq,stack,dir) for layer 0 then layer 1,
  with AllGathers to exchange the per-direction hidden histories between
  layers.  The post-LSTM stages (gating, cross-attention, output transforms)
  are computed per-sequence; the host reads the prefix output from core 0
  and the suffix output from core 1.

All matmuls run in bf16 (f32 PSUM accumulation).  Time-reversal needed by
the bidirectional scans is handled by writing each step's hidden state at
both ascending and descending offsets (H_loc / H_rev) and selecting with
per-core 0/1 mask inputs, keeping the program SPMD-uniform.
"""

import math
import sys

sys.path.insert(0, "/opt/trn_rl_repo")

import ml_dtypes
import numpy as np

import concourse.bass as bass
import concourse.mybir as mybir
import concourse.tile as tile
from concourse.bass_utils import run_bass_kernel_spmd
from concourse.masks import make_identity

BF16 = mybir.dt.bfloat16
F32 = mybir.dt.float32
AF = mybir.ActivationFunctionType
ALU = mybir.AluOpType

C = 1024
T = 64            # prefix_length == suffix_length
H2 = 512
NH = 8
HD = C // NH      # 128
G = 4 * H2        # 2048 lstm gate dim
NC = 8            # cores
KC_IN = C // 128  # 8 input-dim chunks
KC_H = H2 // 128  # 4 hidden-dim chunks
MC_G = G // 128   # 16 gate-dim chunks
HW = KC_H * T     # 256 cols of one hidden history

_CACHE = {}


def _bf(x):
    return np.ascontiguousarray(np.asarray(x, np.float32).astype(ml_dtypes.bfloat16))


def _f32(x):
    return np.ascontiguousarray(np.asarray(x, np.float32))


def _sinusoid(t, c):
    pos = np.arange(t, dtype=np.float32)[:, None]
    div = np.exp((-math.log(10000.0) * np.arange(0, c, 2, dtype=np.float32) / c)
                 .astype(np.float32)).astype(np.float32)
    pe = np.zeros((t, c), np.float32)
    pe[:, 0::2] = np.sin(pos * div)
    pe[:, 1::2] = np.cos(pos * div)
    return pe


def _wtiles(w, kc, mc):
    """w [mc*128, kc*128] -> tiles [128, kc, mc, 128]: t[p,k,m,j] = w[m*128+j, k*128+p]."""
    nm, nk = w.shape
    assert nm == mc * 128 and nk == kc * 128
    out = w.reshape(mc, 128, kc, 128).transpose(3, 2, 0, 1)
    return np.ascontiguousarray(out)


def _wmoving(w, kc):
    """w [n, kc*128] -> [128, kc, n]: out[p,k,n] = w[n, k*128+p]."""
    n = w.shape[0]
    out = w.reshape(n, kc, 128).transpose(2, 1, 0)
    return np.ascontiguousarray(out)


def _pchunk(v):
    """v [m*128] -> [128, m] per-partition chunk layout."""
    m = v.shape[0] // 128
    return np.ascontiguousarray(v.reshape(m, 128).T)


def split_sync_waits(nc):
    """Walrus NO_STRUCT instructions hold limited sem-waits; split extras onto NoOps."""
    limited = (mybir.InstDrain, mybir.InstNoOp)
    fn = nc.m.functions[0]
    for blk in fn.blocks:
        newl = []
        for inst in blk.instructions:
            si = inst.sync_info
            maxw = 1
            if si is not None and len(si.on_wait) > maxw:
                waits = list(si.on_wait)
                pre, keep = waits[:-maxw], waits[-maxw:]
                for i, w in enumerate(pre):
                    nop = mybir.InstNoOp(name=f"{inst.name}-sw{i}", ins=[], outs=[])
                    nop.engine = inst.engine
                    nop.sync_info = mybir.SyncInfo(on_wait=[w], on_update=[])
                    newl.append(nop)
                si.on_wait = keep
                inst.sync_info = si
            newl.append(inst)
        blk.instructions = newl


def build_program():
    nc = bass.Bass()

    def din(name, shape, dt=BF16):
        return nc.dram_tensor(name, shape, dt, kind="ExternalInput")

    embT_d = din("embT", [128, KC_IN, T])
    wih_d = [din(f"wih{l}", [128, KC_IN, MC_G, 128]) for l in range(2)]
    whh_d = [din(f"whh{l}", [128, KC_H, MC_G, 128]) for l in range(2)]
    bias_d = [din(f"bias{l}", [128, MC_G], F32) for l in range(2)]
    selw_d = din("selw", [128, 16], F32)
    mseq_d = din("mseq", [128, 2], F32)
    gw1m_d = din("gw1m", [128, 2 * KC_IN, C])
    gw2s_d = din("gw2s", [128, KC_IN, MC_G, 128])
    gb1r_d = din("gb1r", [64, C])
    glgr_d = din("glgr", [64, C])
    glbr_d = din("glbr", [64, C])
    gb2c_d = din("gb2c", [128, MC_G], F32)
    wqs_d = din("wqs", [128, KC_IN, KC_IN, 128])
    wks_d = din("wks", [128, KC_IN, KC_IN, 128])
    bqc_d = din("bqc", [128, KC_IN], F32)
    bkc_d = din("bkc", [128, KC_IN], F32)
    wvm_d = din("wvm", [128, KC_IN, C])
    bvr_d = din("bvr", [64, C])
    wom_d = din("wom", [128, KC_IN, C])
    bor_d = din("bor", [64, C])
    ow1m_d = din("ow1m", [128, KC_IN, 2 * C])
    ob1r_d = din("ob1r", [64, 2 * C])
    og1r_d = din("og1r", [64, 2 * C])
    obe1r_d = din("obe1r", [64, 2 * C])
    ow2m_d = din("ow2m", [128, 2 * KC_IN, C])
    ob2r_d = din("ob2r", [64, C])
    og2r_d = din("og2r", [64, C])
    obe2r_d = din("obe2r", [64, C])
    lngr_d = din("lngr", [64, C])
    lnbr_d = din("lnbr", [64, C])
    po_d = nc.dram_tensor("po_out", [64, C], F32, kind="ExternalOutput")

    with tile.TileContext(nc) as tc:
        with (
            tc.tile_pool(name="w", bufs=1) as wp,          # persistent constants/state
            tc.tile_pool(name="wih", bufs=2) as wihp,      # streamed lstm input weights
            tc.tile_pool(name="whh", bufs=2) as whhp,      # streamed lstm recurrent weights
            tc.tile_pool(name="pw", bufs=2) as pwp,        # streamed post weights (16KB tiles)
            tc.tile_pool(name="rep", bufs=3) as repp,      # streamed replicated biases
            tc.tile_pool(name="sm", bufs=2) as sm,         # small working tiles
            tc.tile_pool(name="act", bufs=1) as ap,        # activations
            tc.tile_pool(name="ln", bufs=1) as lnp,        # LN scratch
            tc.tile_pool(name="ps", bufs=1, space="PSUM") as ps,
            tc.tile_pool(name="ps2", bufs=2, space="PSUM") as ps2,
            tc.tile_pool(name="dram", bufs=1, space="DRAM") as dram,
        ):
            # ---- constants / inputs to SBUF
            If32 = wp.tile([128, 128], F32)
            make_identity(nc, If32[:])
            Ibf = wp.tile([128, 128], BF16)
            nc.vector.tensor_copy(Ibf[:], If32[:])

            embT = wp.tile([128, KC_IN, T], BF16)
            nc.sync.dma_start(embT[:], embT_d[:])
            selw = wp.tile([128, 16], F32)
            nc.sync.dma_start(selw[:], selw_d[:])
            mseq = wp.tile([128, 2], F32)
            nc.sync.dma_start(mseq[:], mseq_d[:])
            bias = [wp.tile([128, MC_G], F32, tag=f"bias{l}", name=f"bias{l}") for l in range(2)]
            for l in range(2):
                nc.sync.dma_start(bias[l][:], bias_d[l][:])

            whh = [whhp.tile([128, KC_H, MC_G, 128], BF16, tag="whh", name=f"whhl{l}") for l in range(2)]
            for l in range(2):
                nc.sync.dma_start(whh[l][:], whh_d[l][:])

            # state carried across the whole LSTM section
            H = [[wp.tile([128, KC_H, T], BF16, tag=f"H{l}{b}", name=f"H{l}{b}") for b in range(2)]
                 for l in range(2)]  # H[layer][0]=loc, [1]=rev
            xT1 = wp.tile([128, KC_IN, T], BF16)
            ginT = wp.tile([128, 2 * KC_IN, T], BF16)

            ag1_in = dram.tile([128, 2 * HW], BF16, tag="agin")
            ag1_out = dram.tile([NC * 128, 2 * HW], BF16, tag="agout")
            ag2_in = dram.tile([128, 2 * HW], BF16, tag="agin2")
            ag2_out = dram.tile([NC * 128, 2 * HW], BF16, tag="agout2")
            ag3_in = dram.tile([128, 2 * HW], BF16, tag="agin3")
            ag3_out = dram.tile([NC * 128, 2 * HW], BF16, tag="agout3")

            def x_precompute(layer, rhs):
                """X = Wih @ x + biases -> [128, gate-chunk, T] f32 sbuf."""
                xsb = wp.tile([128, MC_G, T], F32, tag="X")
                for half in range(2):
                    wih_h = wihp.tile([128, KC_IN, MC_G // 2, 128], BF16, tag="wih")
                    nc.sync.dma_start(
                        wih_h[:], wih_d[layer][:, :, half * 8:(half + 1) * 8, :])
                    xps = ps.tile([128, 1024], F32, tag="mm")
                    for mc in range(MC_G // 2):
                        m = half * (MC_G // 2) + mc
                        for kc in range(KC_IN):
                            nc.tensor.matmul(
                                xps[:, mc * T:(mc + 1) * T],
                                wih_h[:, kc, mc, :],
                                rhs[:, kc, :],
                                start=(kc == 0), stop=(kc == KC_IN - 1))
                        nc.vector.tensor_scalar_add(
                            xsb[:, m, :],
                            xps[:, mc * T:(mc + 1) * T],
                            bias[layer][:, m:m + 1])
                return xsb

            def lstm_layer(layer, X):
                c_sb = sm.tile([128, KC_H], F32, tag="c_sb")
                h_sb = sm.tile([128, KC_H], BF16, tag="h_sb")
                hloc, hrev = H[layer]
                for i in range(T):
                    z = sm.tile([128, MC_G], F32, tag="z")
                    if i == 0:
                        nc.vector.tensor_copy(z[:], X[:, :, 0])
                    else:
                        zps = ps2.tile([128, MC_G], F32, tag="tp")
                        for mc in range(MC_G):
                            for kc in range(KC_H):
                                nc.tensor.matmul(
                                    zps[:, mc:mc + 1],
                                    whh[layer][:, kc, mc, :],
                                    h_sb[:, kc:kc + 1],
                                    start=(kc == 0), stop=(kc == KC_H - 1))
                        nc.vector.tensor_add(z[:], zps[:], X[:, :, i])
                    sif = sm.tile([128, 8], F32, tag="sif")
                    nc.scalar.activation(sif[:], z[:, 0:8], AF.Sigmoid)
                    tg = sm.tile([128, 4], F32, tag="tg")
                    nc.scalar.activation(tg[:], z[:, 8:12], AF.Tanh)
                    so = sm.tile([128, 4], F32, tag="so")
                    nc.scalar.activation(so[:], z[:, 12:16], AF.Sigmoid)
                    ig = sm.tile([128, 4], F32, tag="ig")
                    nc.vector.tensor_mul(ig[:], sif[:, 0:4], tg[:])
                    if i == 0:
                        nc.vector.tensor_copy(c_sb[:], ig[:])
                    else:
                        nc.vector.tensor_mul(c_sb[:], sif[:, 4:8], c_sb[:])
                        nc.vector.tensor_add(c_sb[:], c_sb[:], ig[:])
                    tc_ = sm.tile([128, 4], F32, tag="tc_")
                    nc.scalar.activation(tc_[:], c_sb[:], AF.Tanh)
                    nc.vector.tensor_mul(h_sb[:], so[:], tc_[:])
                    # write history at ascending and descending offsets
                    nc.vector.tensor_copy(hloc[:, :, i], h_sb[:])
                    nc.vector.tensor_copy(hrev[:, :, T - 1 - i], h_sb[:])

            def allgather(inp_flat, ag_in, ag_out):
                all_sb = wp.tile([128, NC, 2 * HW], BF16, tag="allg")
                nc.sync.dma_start(ag_in[:], inp_flat)
                nc.gpsimd.collective_compute(
                    "AllGather", ALU.bypass,
                    ins=[ag_in.opt()], outs=[ag_out.opt()],
                    replica_groups=[list(range(NC))])
                for r in range(NC):
                    nc.sync.dma_start(all_sb[:, r, :], ag_out[r * 128:(r + 1) * 128, :])
                return all_sb

            # ================= LSTM =================
            X0 = x_precompute(0, embT)
            lstm_layer(0, X0)

            hcat0 = sm.tile([128, 2 * HW], BF16, tag="hcat")
            nc.vector.tensor_copy(hcat0[:, 0:HW], H[0][0][:].rearrange('p a b -> p (a b)'))
            nc.vector.tensor_copy(hcat0[:, HW:], H[0][1][:].rearrange('p a b -> p (a b)'))
            all1 = allgather(hcat0[:], ag1_in, ag1_out)

            # xT1 group A (chunks 0-3) = dir0-core's history, group B = dir1-core's
            xf = xT1[:].rearrange('p a b -> p (a b)')
            for g in range(2):
                dst = xf[:, g * HW:(g + 1) * HW]
                first = True
                for k in range(4):
                    for L in range(2):
                        w_ap = selw[:, g * 8 + k * 2 + L: g * 8 + k * 2 + L + 1]
                        src = all1[:, k + 4 * g, L * HW:(L + 1) * HW]
                        if first:
                            nc.vector.tensor_scalar_mul(dst, src, w_ap)
                            first = False
                        else:
                            nc.vector.scalar_tensor_tensor(
                                out=dst, in0=src, scalar=w_ap, in1=dst,
                                op0=ALU.mult, op1=ALU.add)

            X1 = x_precompute(1, xT1)
            lstm_layer(1, X1)

            hcat1 = sm.tile([128, 2 * HW], BF16, tag="hcat")
            nc.vector.tensor_copy(hcat1[:, 0:HW], H[1][0][:].rearrange('p a b -> p (a b)'))
            nc.vector.tensor_copy(hcat1[:, HW:], H[1][1][:].rearrange('p a b -> p (a b)'))
            all2 = allgather(hcat1[:], ag2_in, ag2_out)

            # ginT: 4 chain groups x 256 cols; chain (stack,dir) sits at rank
            # seq + 2*stack + 4*dir; take loc if stack==dir else rev.
            mp = mseq[:, 0:1]
            ms = mseq[:, 1:2]
            gin_flat = ginT[:].rearrange('p a b -> p (a b)')
            for gi, (st, dr) in enumerate([(0, 0), (0, 1), (1, 0), (1, 1)]):
                L = 0 if st == dr else 1
                base = 2 * st + 4 * dr
                dst = gin_flat[:, gi * HW:(gi + 1) * HW]
                nc.vector.tensor_scalar_mul(dst, all2[:, base, L * HW:(L + 1) * HW], mp)
                nc.vector.scalar_tensor_tensor(
                    out=dst, in0=all2[:, base + 1, L * HW:(L + 1) * HW], scalar=ms,
                    in1=dst, op0=ALU.mult, op1=ALU.add)

            # ---- helpers for post stages
            def ln_(x, gam, bet, out, F):
                s = sm.tile([64, 1], F32, tag="ln_s")
                nc.vector.reduce_sum(s[:], x[:, :F], axis=mybir.AxisListType.X)
                negmu = sm.tile([64, 1], F32, tag="ln_negmu")
                nc.vector.tensor_scalar_mul(negmu[:], s[:], -1.0 / F)
                xc = lnp.tile([64, 2 * C], F32, tag="ln_xc")
                nc.vector.tensor_scalar_add(xc[:, :F], x[:, :F], negmu[:])
                sq = lnp.tile([64, 2 * C], F32, tag="ln_sq")
                ssum = sm.tile([64, 1], F32, tag="ln_ssum")
                nc.vector.scalar_tensor_tensor(
                    out=sq[:, :F], in0=xc[:, :F], scalar=1.0, in1=xc[:, :F],
                    op0=ALU.mult, op1=ALU.mult, accum_out=ssum[:])
                var = sm.tile([64, 1], F32, tag="ln_var")
                nc.vector.tensor_scalar_mul(var[:], ssum[:], 1.0 / F)
                sd = sm.tile([64, 1], F32, tag="ln_sd")
                nc.scalar.activation(sd[:], var[:], AF.Sqrt, bias=eps_t[:])
                rstd = sm.tile([64, 1], F32, tag="ln_rstd")
                nc.vector.reciprocal(rstd[:], sd[:])
                nc.vector.tensor_scalar_mul(xc[:, :F], xc[:, :F], rstd[:])
                nc.vector.tensor_mul(xc[:, :F], xc[:, :F], gam[:, :F])
                nc.vector.tensor_add(out[:, :F], xc[:, :F], bet[:, :F])

            def rep_tile(d, F):
                t = repp.tile([64, 2 * C], BF16, tag="rep")
                nc.sync.dma_start(t[:, :F], d[:])
                return t

            def transpose_to(dstT, src_nat, nchunks, ident, tagsuffix=""):
                """src_nat [64, nchunks*128] -> dstT [128, nchunks, T] bf16."""
                for kc in range(nchunks):
                    tp = ps2.tile([128, 64], F32, tag="tp")
                    nc.tensor.transpose(tp[:], src_nat[:, kc * 128:(kc + 1) * 128],
                                        ident[:64, :64])
                    nc.vector.tensor_copy(dstT[:, kc, :], tp[:])

            eps_t = wp.tile([64, 1], F32)
            nc.gpsimd.memset(eps_t[:], 1e-5)
            natA = ap.tile([64, 2 * C], F32, tag="natA")
            natB = ap.tile([64, 2 * C], F32, tag="natB")

            # ================= gate stage =================
            gps = ps.tile([64, 2 * C], F32, tag="mm")
            for ns in range(2):
                gw1_h = pwp.tile([128, 2 * KC_IN, 512], BF16, tag="big")
                nc.sync.dma_start(gw1_h[:], gw1m_d[:, :, ns * 512:(ns + 1) * 512])
                for kc in range(2 * KC_IN):
                    nc.tensor.matmul(
                        gps[:, ns * 512:(ns + 1) * 512],
                        ginT[:, kc, :],
                        gw1_h[:, kc, :],
                        start=(kc == 0), stop=(kc == 2 * KC_IN - 1))
            gb1r = rep_tile(gb1r_d, C)
            nc.vector.tensor_add(natA[:, :C], gps[:, :C], gb1r[:, :C])
            glgr = rep_tile(glgr_d, C)
            glbr = rep_tile(glbr_d, C)
            ln_(natA, glgr, glbr, natB, C)
            nc.scalar.activation(natA[:, :C], natB[:, :C], AF.Gelu)

            g_hT = ap.tile([128, KC_IN, T], BF16, tag="g_hT")
            transpose_to(g_hT, natA, KC_IN, If32)

            gb2c = wp.tile([128, MC_G], F32)
            nc.sync.dma_start(gb2c[:], gb2c_d[:])
            gatesT = ap.tile([128, MC_G, T], BF16, tag="gatesT")
            for half in range(2):
                gw2_h = pwp.tile([128, KC_IN, MC_G // 2, 128], BF16, tag="big")
                nc.sync.dma_start(gw2_h[:], gw2s_d[:, :, half * 8:(half + 1) * 8, :])
                gtv = ps.tile([128, 1024], F32, tag="mm")
                for mc in range(MC_G // 2):
                    m = half * (MC_G // 2) + mc
                    for kc in range(KC_IN):
                        nc.tensor.matmul(
                            gtv[:, mc * T:(mc + 1) * T],
                            gw2_h[:, kc, mc, :],
                            g_hT[:, kc, :],
                            start=(kc == 0), stop=(kc == KC_IN - 1))
                    nc.scalar.activation(
                        gatesT[:, m, :], gtv[:, mc * T:(mc + 1) * T],
                        AF.Sigmoid, bias=gb2c[:, m:m + 1])

            biT = ap.tile([128, KC_IN, T], BF16, tag="biT")
            gflat = gatesT[:].rearrange('p a b -> p (a b)')
            ginf = ginT[:].rearrange('p a b -> p (a b)')
            bif = biT[:].rearrange('p a b -> p (a b)')
            t1 = sm.tile([128, KC_IN * T], BF16, tag="bi_t1")
            nc.vector.tensor_mul(t1[:], gflat[:, 0:512], ginf[:, 0:512])
            nc.vector.tensor_mul(bif[:], gflat[:, 512:1024], ginf[:, 512:1024])
            nc.vector.tensor_add(bif[:], bif[:], t1[:])

            # exchange biT between sequences (pad AG buffer reuse shape)
            all3 = allgather(bif[:], ag3_in, ag3_out)
            kvT = ap.tile([128, KC_IN, T], BF16, tag="kvT")
            kvf = kvT[:].rearrange('p a b -> p (a b)')
            nc.vector.tensor_scalar_mul(kvf[:], all3[:, 1, 0:512], mp)
            nc.vector.scalar_tensor_tensor(
                out=kvf[:], in0=all3[:, 0, 0:512], scalar=ms, in1=kvf[:],
                op0=ALU.mult, op1=ALU.add)

            # ================= attention =================
            bqc = wp.tile([128, KC_IN], F32)
            nc.sync.dma_start(bqc[:], bqc_d[:])
            bkc = wp.tile([128, KC_IN], F32)
            nc.sync.dma_start(bkc[:], bkc_d[:])

            qT = ap.tile([128, KC_IN, T], BF16, tag="qT")
            kT = ap.tile([128, KC_IN, T], BF16, tag="kT")
            for (dst, wsd, bc, scale, rhs) in (
                (qT, wqs_d, bqc, 1.0 / math.sqrt(HD), biT),
                (kT, wks_d, bkc, 1.0, kvT),
            ):
                ws = pwp.tile([128, KC_IN, KC_IN, 128], BF16, tag="big")
                nc.sync.dma_start(ws[:], wsd[:])
                qv = ps.tile([128, 1024], F32, tag="mm")
                for mc in range(KC_IN):
                    for kc in range(KC_IN):
                        nc.tensor.matmul(
                            qv[:, mc * T:(mc + 1) * T],
                            ws[:, kc, mc, :],
                            rhs[:, kc, :],
                            start=(kc == 0), stop=(kc == KC_IN - 1))
                    nc.vector.tensor_scalar(
                        out=dst[:, mc, :], in0=qv[:, mc * T:(mc + 1) * T],
                        scalar1=bc[:, mc:mc + 1], scalar2=scale,
                        op0=ALU.add, op1=ALU.mult)

            wvm = pwp.tile([128, KC_IN, C], BF16, tag="big")
            nc.sync.dma_start(wvm[:], wvm_d[:])
            vps = ps.tile([64, 2 * C], F32, tag="mm")
            for ns in range(2):
                for kc in range(KC_IN):
                    nc.tensor.matmul(
                        vps[:, ns * 512:(ns + 1) * 512],
                        kvT[:, kc, :],
                        wvm[:, kc, ns * 512:(ns + 1) * 512],
                        start=(kc == 0), stop=(kc == KC_IN - 1))
            bvr = rep_tile(bvr_d, C)
            v_sb = ap.tile([64, C], BF16, tag="v_sb")
            nc.vector.tensor_add(v_sb[:], vps[:, :C], bvr[:, :C])

            sps = ps2.tile([64, NH, T], F32, tag="tp")
            for h in range(NH):
                nc.tensor.matmul(sps[:, h, :], qT[:, h, :], kT[:, h, :],
                                 start=True, stop=True)
            negmax = sm.tile([64, NH], F32, tag="negmax")
            nc.vector.tensor_reduce(negmax[:], sps[:], axis=mybir.AxisListType.X,
                                    op=ALU.max, negate=True)
            attn = ap.tile([64, NH, T], F32, tag="attn")
            sumexp = sm.tile([64, NH], F32, tag="sumexp")
            for h in range(NH):
                nc.scalar.activation(attn[:, h, :], sps[:, h, :], AF.Exp,
                                     bias=negmax[:, h:h + 1],
                                     accum_out=sumexp[:, h:h + 1])
            recip = sm.tile([64, NH], F32, tag="recip")
            nc.vector.reciprocal(recip[:], sumexp[:])

            attnT = ap.tile([64, NH, T], BF16, tag="attnT")
            for h in range(NH):
                tpa = ps2.tile([64, 64], F32, tag="tp")
                nc.tensor.transpose(tpa[:], attn[:, h, :], If32[:64, :64])
                nc.vector.tensor_copy(attnT[:, h, :], tpa[:])

            aops = ps.tile([64, 2 * C], F32, tag="mm")
            for h in range(NH):
                nc.tensor.matmul(aops[:, h * HD:(h + 1) * HD],
                                 attnT[:, h, :], v_sb[:, h * HD:(h + 1) * HD],
                                 start=True, stop=True)
            ao = ap.tile([64, C], BF16, tag="ao")
            for h in range(NH):
                nc.vector.tensor_scalar_mul(
                    ao[:, h * HD:(h + 1) * HD], aops[:, h * HD:(h + 1) * HD],
                    recip[:, h:h + 1])
            aoT = ap.tile([128, KC_IN, T], BF16, tag="aoT")
            for kc in range(KC_IN):
                tpb = ps2.tile([128, 64], BF16, tag="tpb")
                nc.tensor.transpose(tpb[:], ao[:, kc * 128:(kc + 1) * 128], Ibf[:64, :64])
                nc.vector.tensor_copy(aoT[:, kc, :], tpb[:])

            wom = pwp.tile([128, KC_IN, C], BF16, tag="big")
            nc.sync.dma_start(wom[:], wom_d[:])
            paps = ps.tile([64, 2 * C], F32, tag="mm")
            for ns in range(2):
                for kc in range(KC_IN):
                    nc.tensor.matmul(
                        paps[:, ns * 512:(ns + 1) * 512],
                        aoT[:, kc, :],
                        wom[:, kc, ns * 512:(ns + 1) * 512],
                        start=(kc == 0), stop=(kc == KC_IN - 1))

            # residual: bi natural + pa + bo
            nc.vector.tensor_copy(natA[:, :C], paps[:, :C])
            for kc in range(KC_IN):
                tpr = ps2.tile([64, 128], BF16, tag="tpb")
                nc.tensor.transpose(tpr[:], biT[:, kc, :], Ibf[:, :])
                nc.vector.tensor_add(natA[:, kc * 128:(kc + 1) * 128],
                                     natA[:, kc * 128:(kc + 1) * 128], tpr[:])
            bor = rep_tile(bor_d, C)
            nc.vector.tensor_add(natA[:, :C], natA[:, :C], bor[:, :C])

            lngr = rep_tile(lngr_d, C)
            lnbr = rep_tile(lnbr_d, C)
            ln_(natA, lngr, lnbr, natB, C)   # natB = po_pre

            # ================= output transform =================
            ppT = ap.tile([128, KC_IN, T], BF16, tag="ppT")
            transpose_to(ppT, natB, KC_IN, If32)

            h1ps = ps.tile([64, 2 * C], F32, tag="mm")
            for half in range(2):
                ow1_h = pwp.tile([128, KC_IN, C], BF16, tag="big")
                nc.sync.dma_start(ow1_h[:], ow1m_d[:, :, half * C:(half + 1) * C])
                for ns in range(2):
                    for kc in range(KC_IN):
                        nc.tensor.matmul(
                            h1ps[:, half * C + ns * 512: half * C + (ns + 1) * 512],
                            ppT[:, kc, :],
                            ow1_h[:, kc, ns * 512:(ns + 1) * 512],
                            start=(kc == 0), stop=(kc == KC_IN - 1))
            ob1r = rep_tile(ob1r_d, 2 * C)
            nc.vector.tensor_add(natA[:], h1ps[:], ob1r[:])
            og1r = rep_tile(og1r_d, 2 * C)
            obe1r = rep_tile(obe1r_d, 2 * C)
            ln_(natA, og1r, obe1r, natB, 2 * C)
            nc.scalar.activation(natA[:], natB[:], AF.Gelu)

            h1T = ap.tile([128, 2 * KC_IN, T], BF16, tag="h1T")
            transpose_to(h1T, natA, 2 * KC_IN, If32)

            pops = ps.tile([64, 2 * C], F32, tag="mm")
            for half in range(2):
                ow2_h = pwp.tile([128, 2 * KC_IN, 512], BF16, tag="big")
                nc.sync.dma_start(ow2_h[:], ow2m_d[:, :, half * 512:(half + 1) * 512])
                for kc in range(2 * KC_IN):
                    nc.tensor.matmul(
                        pops[:, half * 512:(half + 1) * 512],
                        h1T[:, kc, :],
                        ow2_h[:, kc, :],
                        start=(kc == 0), stop=(kc == 2 * KC_IN - 1))
            ob2r = rep_tile(ob2r_d, C)
            nc.vector.tensor_add(natA[:, :C], pops[:, :C], ob2r[:, :C])
            og2r = rep_tile(og2r_d, C)
            obe2r = rep_tile(obe2r_d, C)
            ln_(natA, og2r, obe2r, natB, C)
            nc.sync.dma_start(po_d[:], natB[:, :C])

    split_sync_waits(nc)
    return nc


def _prep_core_inputs(c, emb_pe, lstm_Wih, lstm_Whh, lstm_bih, lstm_bhh,
                      attn_w, attn_b, gate_w1, gate_b1, gate_lg, gate_lb,
                      gate_w2, gate_b2, out_w1, out_b1, out_g1, out_be1,
                      out_w2, out_b2, out_g2, out_be2, ln_g, ln_b):
    seq = c & 1
    stack = (c >> 1) & 1
    dr = (c >> 2) & 1
    asc = (stack == dr)

    x = emb_pe[seq]
    order = np.arange(T) if asc else np.arange(T)[::-1]
    xl = x[order]  # local time
    embT = _bf(xl.reshape(T, KC_IN, 128).transpose(2, 1, 0))  # [128, kc, t]

    m = {"embT": embT}
    for l in range(2):
        m[f"wih{l}"] = _bf(_wtiles(np.asarray(lstm_Wih[stack, l, dr]), KC_IN, MC_G))
        m[f"whh{l}"] = _bf(_wtiles(np.asarray(lstm_Whh[stack, l, dr]), KC_H, MC_G))
        m[f"bias{l}"] = _f32(_pchunk(np.asarray(lstm_bih[stack, l, dr])
                                     + np.asarray(lstm_bhh[stack, l, dr])))

    selw = np.zeros((16,), np.float32)
    k = seq + 2 * stack
    selw[0 * 8 + k * 2 + dr] = 1.0            # group A: dir0-core, L = mydir
    selw[1 * 8 + k * 2 + (1 - dr)] = 1.0      # group B: dir1-core, L = 1-mydir
    m["selw"] = _f32(np.broadcast_to(selw, (128, 16)))
    mseq = np.zeros((2,), np.float32)
    mseq[seq] = 1.0
    m["mseq"] = _f32(np.broadcast_to(mseq, (128, 2)))

    q = seq
    rep = lambda v: _bf(np.broadcast_to(np.asarray(v, np.float32).reshape(1, -1),
                                        (64, np.asarray(v).shape[-1])))
    m["gw1m"] = _bf(_wmoving(np.asarray(gate_w1[q]), 2 * KC_IN))
    m["gb1r"] = rep(gate_b1[q])
    m["glgr"] = rep(gate_lg[q])
    m["glbr"] = rep(gate_lb[q])
    m["gw2s"] = _bf(_wtiles(np.asarray(gate_w2[q]), KC_IN, MC_G))
    m["gb2c"] = _f32(_pchunk(np.asarray(gate_b2[q])))
    m["wqs"] = _bf(_wtiles(np.asarray(attn_w[q, 0]), KC_IN, KC_IN))
    m["wks"] = _bf(_wtiles(np.asarray(attn_w[q, 1]), KC_IN, KC_IN))
    m["bqc"] = _f32(_pchunk(np.asarray(attn_b[q, 0])))
    m["bkc"] = _f32(_pchunk(np.asarray(attn_b[q, 1])))
    m["wvm"] = _bf(_wmoving(np.asarray(attn_w[q, 2]), KC_IN))
    m["bvr"] = rep(attn_b[q, 2])
    m["wom"] = _bf(_wmoving(np.asarray(attn_w[q, 3]), KC_IN))
    m["bor"] = rep(attn_b[q, 3])
    m["ow1m"] = _bf(_wmoving(np.asarray(out_w1[q]), KC_IN))
    m["ob1r"] = rep(out_b1[q])
    m["og1r"] = rep(out_g1[q])
    m["obe1r"] = rep(out_be1[q])
    m["ow2m"] = _bf(_wmoving(np.asarray(out_w2[q]), 2 * KC_IN))
    m["ob2r"] = rep(out_b2[q])
    m["og2r"] = rep(out_g2[q])
    m["obe2r"] = rep(out_be2[q])
    m["lngr"] = rep(ln_g)
    m["lnbr"] = rep(ln_b)
    return m


def _get_runner():
    """Build the program once and wrap it in a persistently-jitted SPMD callable."""
    if "runner" in _CACHE:
        return _CACHE["runner"]

    import jax
    from jax.sharding import Mesh, PartitionSpec
    from jax.experimental.shard_map import shard_map
    from concourse import bass2jax

    nc = build_program()
    bass2jax.install_neuronx_cc_hook()

    in_names, out_names, out_avals, zero_outs = [], [], [], []
    for alloc in nc.m.functions[0].allocations:
        if not isinstance(alloc, mybir.MemoryLocationSet):
            continue
        name = alloc.memorylocations[0].name
        pname = nc.partition_id_tensor.name if nc.partition_id_tensor else None
        if alloc.kind == "ExternalInput":
            if name != pname:
                in_names.append(name)
        elif alloc.kind == "ExternalOutput":
            shape = tuple(alloc.tensor_shape)
            dtype = mybir.dt.np(alloc.dtype)
            out_names.append(name)
            out_avals.append(jax.core.ShapedArray(shape, dtype))
            zero_outs.append(np.zeros(shape, dtype))
    n_params = len(in_names)
    all_in = in_names + out_names
    donate = tuple(range(n_params, n_params + len(out_names)))

    def _body(*args):
        operands = list(args)
        if nc.partition_id_tensor is not None:
            operands.append(bass2jax.partition_id_tensor())
        outs = bass2jax._bass_exec_p.bind(
            *operands,
            out_avals=tuple(out_avals),
            in_names=tuple(all_in + ([nc.partition_id_tensor.name]
                                     if nc.partition_id_tensor else [])),
            out_names=tuple(out_names),
            lowering_input_output_aliases=(),
            sim_require_finite=True,
            sim_require_nnan=True,
            nc=nc,
        )
        return tuple(outs)

    devices = jax.devices()[:NC]
    mesh = Mesh(np.asarray(devices), ("core",))
    pspec = (PartitionSpec("core"),)
    sharded = jax.jit(
        shard_map(_body, mesh=mesh,
                  in_specs=pspec * (n_params + len(out_names)),
                  out_specs=pspec * len(out_names), check_rep=False),
        donate_argnums=donate, keep_unused=True)

    def run(in_maps):
        concat_in = [np.concatenate([np.asarray(in_maps[c][nm])
                                     for c in range(NC)], axis=0)
                     for nm in in_names]
        concat_zero = [np.zeros((NC * z.shape[0], *z.shape[1:]), z.dtype)
                       for z in zero_outs]
        out_arrs = sharded(*concat_in, *concat_zero)
        return [
            {nm: np.asarray(out_arrs[i]).reshape(NC, *out_avals[i].shape)[c]
             for i, nm in enumerate(out_names)}
            for c in range(NC)
        ]

    run.sharded = sharded
    run.in_names = in_names
    run.out_names = out_names
    run.out_avals = out_avals
    run.zero_outs = zero_outs
    _CACHE["runner"] = run
    return run


def kernel(prefix_emb, suffix_emb, lstm_Wih, lstm_Whh, lstm_bih, lstm_bhh,
           attn_w, attn_b, gate_w1, gate_b1, gate_lg, gate_lb, gate_w2, gate_b2,
           out_w1, out_b1, out_g1, out_be1, out_w2, out_b2, out_g2, out_be2,
           ln_g, ln_b, batch_size):
    run = _get_runner()

    pe = _sinusoid(T, C)
    emb_pe = (np.asarray(prefix_emb, np.float32) + pe,
              np.asarray(suffix_emb, np.float32) + pe)

    in_maps = [
        _prep_core_inputs(c, emb_pe, lstm_Wih, lstm_Whh, lstm_bih, lstm_bhh,
                          attn_w, attn_b, gate_w1, gate_b1, gate_lg, gate_lb,
                          gate_w2, gate_b2, out_w1, out_b1, out_g1, out_be1,
                          out_w2, out_b2, out_g2, out_be2,
                          np.asarray(ln_g, np.float32),
                          np.asarray(ln_b, np.float32))
        for c in range(NC)
    ]
    results = run(in_maps)
    po = np.asarray(results[0]["po_out"], np.float32)
    so = np.asarray(results[1]["po_out"], np.float32)
    b = int(batch_size)
    po_b = np.broadcast_to(po[None], (b, T, C)).copy()
    so_b = np.broadcast_to(so[None], (b, T, C)).copy()
    return po_b, so_b


# revision 22
# speedup vs baseline: 16950.4960x; 218.1588x over previous
"""Trainium2 Bass kernel for BidirectionalAttentionalPromptEncoder.

Key algebraic fact: every batch element of the reference is IDENTICAL
(the input embeddings are broadcast over batch before any compute), so we
compute a single batch element on-device and broadcast on the host.

Distribution (8 NeuronCores, SPMD single program, data-driven per-core roles):
  core c:  seq = c&1 (0=prefix,1=suffix), stack = (c>>1)&1 (forward/backward
  LSTM stack), dir = (c>>2)&1 (direction inside the bidirectional layer).
  Each core runs one LSTM chain (seq,stack,dir) for layer 0 then layer 1,
  with AllGathers to exchange the per-direction hidden histories between
  layers.  The post-LSTM stages (gating, cross-attention, output transforms)
  are computed per-sequence; the host reads the prefix output from core 0
  and the suffix output from core 1.

All matmuls run in bf16 (f32 PSUM accumulation).  Time-reversal needed by
the bidirectional scans is handled by writing each step's hidden state at
both ascending and descending offsets (H_loc / H_rev) and selecting with
per-core 0/1 mask inputs, keeping the program SPMD-uniform.
"""

import math
import sys

sys.path.insert(0, "/opt/trn_rl_repo")

import ml_dtypes
import numpy as np

import concourse.bass as bass
import concourse.mybir as mybir
import concourse.tile as tile
from concourse.bass_utils import run_bass_kernel_spmd
from concourse.masks import make_identity

BF16 = mybir.dt.bfloat16
F32 = mybir.dt.float32
AF = mybir.ActivationFunctionType
ALU = mybir.AluOpType

C = 1024
T = 64            # prefix_length == suffix_length
H2 = 512
NH = 8
HD = C // NH      # 128
G = 4 * H2        # 2048 lstm gate dim
NC = 8            # cores
KC_IN = C // 128  # 8 input-dim chunks
KC_H = H2 // 128  # 4 hidden-dim chunks
MC_G = G // 128   # 16 gate-dim chunks
HW = KC_H * T     # 256 cols of one hidden history

_CACHE = {}


def _bf(x):
    return np.ascontiguousarray(np.asarray(x, np.float32).astype(ml_dtypes.bfloat16))


def _f32(x):
    return np.ascontiguousarray(np.asarray(x, np.float32))


def _sinusoid(t, c):
    pos = np.arange(t, dtype=np.float32)[:, None]
    div = np.exp((-math.log(10000.0) * np.arange(0, c, 2, dtype=np.float32) / c)
                 .astype(np.float32)).astype(np.float32)
    pe = np.zeros((t, c), np.float32)
    pe[:, 0::2] = np.sin(pos * div)
    pe[:, 1::2] = np.cos(pos * div)
    return pe


def _wtiles(w, kc, mc):
    """w [mc*128, kc*128] -> bf16 tiles [128, kc, mc, 128]: t[p,k,m,j] = w[m*128+j, k*128+p]."""
    wb = np.asarray(w, np.float32).astype(ml_dtypes.bfloat16)
    out = wb.reshape(mc, 128, kc, 128).transpose(3, 2, 0, 1)
    return np.ascontiguousarray(out)


def _wmoving(w, kc):
    """w [n, kc*128] -> bf16 [128, kc, n]: out[p,k,n] = w[n, k*128+p]."""
    wb = np.asarray(w, np.float32).astype(ml_dtypes.bfloat16)
    n = wb.shape[0]
    out = wb.reshape(n, kc, 128).transpose(2, 1, 0)
    return np.ascontiguousarray(out)


def _pchunk(v):
    """v [m*128] -> [128, m] per-partition chunk layout."""
    m = v.shape[0] // 128
    return np.ascontiguousarray(v.reshape(m, 128).T)


def split_sync_waits(nc):
    """Walrus NO_STRUCT instructions hold limited sem-waits; split extras onto NoOps."""
    limited = (mybir.InstDrain, mybir.InstNoOp)
    fn = nc.m.functions[0]
    for blk in fn.blocks:
        newl = []
        for inst in blk.instructions:
            si = inst.sync_info
            maxw = 1
            if si is not None and len(si.on_wait) > maxw:
                waits = list(si.on_wait)
                pre, keep = waits[:-maxw], waits[-maxw:]
                for i, w in enumerate(pre):
                    nop = mybir.InstNoOp(name=f"{inst.name}-sw{i}", ins=[], outs=[])
                    nop.engine = inst.engine
                    nop.sync_info = mybir.SyncInfo(on_wait=[w], on_update=[])
                    newl.append(nop)
                si.on_wait = keep
                inst.sync_info = si
            newl.append(inst)
        blk.instructions = newl


def build_program():
    nc = bass.Bass()

    def din(name, shape, dt=BF16):
        return nc.dram_tensor(name, shape, dt, kind="ExternalInput")

    embT_d = din("embT", [128, KC_IN, T])
    wih_d = [din(f"wih{l}", [128, KC_IN, MC_G, 128]) for l in range(2)]
    whh_d = [din(f"whh{l}", [128, KC_H, MC_G, 128]) for l in range(2)]
    bias_d = [din(f"bias{l}", [128, MC_G], F32) for l in range(2)]
    selw_d = din("selw", [128, 16], F32)
    mseq_d = din("mseq", [128, 4], F32)
    gw1m_d = din("gw1m", [128, 2 * KC_IN, C])
    gw2s_d = din("gw2s", [128, KC_IN, MC_G, 128])
    gb1r_d = din("gb1r", [64, C])
    glgr_d = din("glgr", [64, C])
    glbr_d = din("glbr", [64, C])
    gb2c_d = din("gb2c", [128, MC_G], F32)
    wqs_d = din("wqs", [128, KC_IN, KC_IN, 128])
    wks_d = din("wks", [128, KC_IN, KC_IN, 128])
    bqc_d = din("bqc", [128, KC_IN], F32)
    bkc_d = din("bkc", [128, KC_IN], F32)
    wvm_d = din("wvm", [128, KC_IN, C])
    bvr_d = din("bvr", [64, C])
    wom_d = din("wom", [128, KC_IN, C])
    bor_d = din("bor", [64, C])
    ow1m_d = din("ow1m", [128, KC_IN, 2 * C])
    ob1r_d = din("ob1r", [64, 2 * C])
    og1r_d = din("og1r", [64, 2 * C])
    obe1r_d = din("obe1r", [64, 2 * C])
    ow2m_d = din("ow2m", [128, 2 * KC_IN, C])
    ob2r_d = din("ob2r", [64, C])
    og2r_d = din("og2r", [64, C])
    obe2r_d = din("obe2r", [64, C])
    lngr_d = din("lngr", [64, C])
    lnbr_d = din("lnbr", [64, C])
    po_d = nc.dram_tensor("po_out", [64, C], F32, kind="ExternalOutput")

    with tile.TileContext(nc) as tc:
        with (
            tc.tile_pool(name="w", bufs=1) as wp,          # persistent constants/state
            tc.tile_pool(name="wih", bufs=2) as wihp,      # streamed lstm input weights
            tc.tile_pool(name="whh", bufs=2) as whhp,      # streamed lstm recurrent weights
            tc.tile_pool(name="pw", bufs=2) as pwp,        # streamed post weights (16KB tiles)
            tc.tile_pool(name="rep", bufs=3) as repp,      # streamed replicated biases
            tc.tile_pool(name="sm", bufs=2) as sm,         # small working tiles
            tc.tile_pool(name="act", bufs=1) as ap,        # activations
            tc.tile_pool(name="ln", bufs=1) as lnp,        # LN scratch
            tc.tile_pool(name="ps", bufs=1, space="PSUM") as ps,
            tc.tile_pool(name="ps2", bufs=2, space="PSUM") as ps2,
            tc.tile_pool(name="dram", bufs=1, space="DRAM") as dram,
        ):
            # ---- constants / inputs to SBUF
            If32 = wp.tile([128, 128], F32)
            make_identity(nc, If32[:])
            Ibf = wp.tile([128, 128], BF16)
            nc.vector.tensor_copy(Ibf[:], If32[:])

            embT = wp.tile([128, KC_IN, T], BF16)
            nc.sync.dma_start(embT[:], embT_d[:])
            selw = wp.tile([128, 16], F32)
            nc.sync.dma_start(selw[:], selw_d[:])
            mseq = wp.tile([128, 4], F32)
            nc.sync.dma_start(mseq[:], mseq_d[:])
            bias = [wp.tile([128, MC_G], F32, tag=f"bias{l}", name=f"bias{l}") for l in range(2)]
            for l in range(2):
                nc.sync.dma_start(bias[l][:], bias_d[l][:])

            whh = [whhp.tile([128, KC_H, MC_G, 128], BF16, tag="whh", name=f"whhl{l}") for l in range(2)]
            for l in range(2):
                nc.sync.dma_start(whh[l][:], whh_d[l][:])

            # state carried across the whole LSTM section
            H = [[wp.tile([128, KC_H, T], BF16, tag=f"H{l}{b}", name=f"H{l}{b}") for b in range(2)]
                 for l in range(2)]  # H[layer][0]=loc, [1]=rev
            xT1 = wp.tile([128, KC_IN, T], BF16)
            ginT = wp.tile([128, 2 * KC_IN, T], BF16)

            ag1_in = dram.tile([128, HW], BF16, tag="agin")
            ag1_out = dram.tile([2 * 128, HW], BF16, tag="agout")
            ag2_in = dram.tile([128, HW], BF16, tag="agin2")
            ag2_out = dram.tile([4 * 128, HW], BF16, tag="agout2")
            ag3_in = dram.tile([128, 2 * HW], BF16, tag="agin3")
            ag3_out = dram.tile([2 * 128, 2 * HW], BF16, tag="agout3")

            def x_precompute(layer, rhs):
                """X = Wih @ x + biases -> [128, gate-chunk, T] f32 sbuf."""
                xsb = wp.tile([128, MC_G, T], F32, tag="X")
                for half in range(2):
                    wih_h = wihp.tile([128, KC_IN, MC_G // 2, 128], BF16, tag="wih")
                    nc.sync.dma_start(
                        wih_h[:], wih_d[layer][:, :, half * 8:(half + 1) * 8, :])
                    xps = ps.tile([128, 1024], F32, tag="mm")
                    for mc in range(MC_G // 2):
                        m = half * (MC_G // 2) + mc
                        for kc in range(KC_IN):
                            nc.tensor.matmul(
                                xps[:, mc * T:(mc + 1) * T],
                                wih_h[:, kc, mc, :],
                                rhs[:, kc, :],
                                start=(kc == 0), stop=(kc == KC_IN - 1))
                        nc.vector.tensor_scalar_add(
                            xsb[:, m, :],
                            xps[:, mc * T:(mc + 1) * T],
                            bias[layer][:, m:m + 1])
                return xsb

            def lstm_layer(layer, X):
                c_sb = sm.tile([128, KC_H], F32, tag="c_sb")
                h_sb = sm.tile([128, KC_H], BF16, tag="h_sb")
                hloc, hrev = H[layer]
                for i in range(T):
                    z = sm.tile([128, MC_G], F32, tag="z")
                    if i == 0:
                        nc.vector.tensor_copy(z[:], X[:, :, 0])
                    else:
                        zps = ps2.tile([128, MC_G], F32, tag="tp")
                        for mc in range(MC_G):
                            for kc in range(KC_H):
                                nc.tensor.matmul(
                                    zps[:, mc:mc + 1],
                                    whh[layer][:, kc, mc, :],
                                    h_sb[:, kc:kc + 1],
                                    start=(kc == 0), stop=(kc == KC_H - 1))
                        nc.vector.tensor_add(z[:], zps[:], X[:, :, i])
                    sif = sm.tile([128, 8], F32, tag="sif")
                    nc.scalar.activation(sif[:], z[:, 0:8], AF.Sigmoid)
                    tg = sm.tile([128, 4], F32, tag="tg")
                    nc.scalar.activation(tg[:], z[:, 8:12], AF.Tanh)
                    so = sm.tile([128, 4], F32, tag="so")
                    nc.scalar.activation(so[:], z[:, 12:16], AF.Sigmoid)
                    ig = sm.tile([128, 4], F32, tag="ig")
                    nc.vector.tensor_mul(ig[:], sif[:, 0:4], tg[:])
                    if i == 0:
                        nc.vector.tensor_copy(c_sb[:], ig[:])
                    else:
                        nc.vector.tensor_mul(c_sb[:], sif[:, 4:8], c_sb[:])
                        nc.vector.tensor_add(c_sb[:], c_sb[:], ig[:])
                    tc_ = sm.tile([128, 4], F32, tag="tc_")
                    nc.scalar.activation(tc_[:], c_sb[:], AF.Tanh)
                    nc.vector.tensor_mul(h_sb[:], so[:], tc_[:])
                    # write history at ascending and descending offsets
                    nc.vector.tensor_copy(hloc[:, :, i], h_sb[:])
                    nc.vector.tensor_copy(hrev[:, :, T - 1 - i], h_sb[:])

            def allgather(inp_flat, ag_in, ag_out, groups, nsl, width, name):
                all_sb = wp.tile([128, nsl, width], BF16, tag="allg", name=name)
                nc.sync.dma_start(ag_in[:], inp_flat)
                nc.gpsimd.collective_compute(
                    "AllGather", ALU.bypass,
                    ins=[ag_in.opt()], outs=[ag_out.opt()],
                    replica_groups=groups)
                for r in range(nsl):
                    nc.sync.dma_start(all_sb[:, r, :], ag_out[r * 128:(r + 1) * 128, :])
                return all_sb

            PAIR_DIR = [[c, c + 4] for c in range(4)]      # (seq,stack) pairs across dir
            PAIR_SEQ = [[2 * c, 2 * c + 1] for c in range(4)]  # across seq
            PARITY = [[0, 2, 4, 6], [1, 3, 5, 7]]

            # ================= LSTM =================
            X0 = x_precompute(0, embT)
            lstm_layer(0, X0)

            all1 = allgather(H[0][1][:].rearrange('p a b -> p (a b)'),
                             ag1_in, ag1_out, PAIR_DIR, 2, HW, "all1sb")

            # xT1 group A (chunks 0-3) = dir0-chain's history in my local time,
            # group B = dir1-chain's.  Sources: own H0loc, or pair slice 0/1
            # (rev of ranks c&3 / (c&3)+4); 0/1 weights from selw cols:
            #   [0]=A_loc [1]=A_sl0 [2]=A_sl1 [3]=B_loc [4]=B_sl0 [5]=B_sl1
            h0loc = H[0][0][:].rearrange('p a b -> p (a b)')
            xf = xT1[:].rearrange('p a b -> p (a b)')
            for g in range(2):
                dst = xf[:, g * HW:(g + 1) * HW]
                nc.vector.tensor_scalar_mul(dst, h0loc[:], selw[:, 3 * g:3 * g + 1])
                for sl in range(2):
                    nc.vector.scalar_tensor_tensor(
                        out=dst, in0=all1[:, sl, :],
                        scalar=selw[:, 3 * g + 1 + sl:3 * g + 2 + sl], in1=dst,
                        op0=ALU.mult, op1=ALU.add)

            X1 = x_precompute(1, xT1)
            lstm_layer(1, X1)

            # each core sends its chain's SOURCE-time history: loc if asc else rev
            h1src = sm.tile([128, HW], BF16, tag="h1src")
            nc.vector.tensor_scalar_mul(
                h1src[:], H[1][0][:].rearrange('p a b -> p (a b)'), mseq[:, 2:3])
            nc.vector.scalar_tensor_tensor(
                out=h1src[:], in0=H[1][1][:].rearrange('p a b -> p (a b)'),
                scalar=mseq[:, 3:4], in1=h1src[:], op0=ALU.mult, op1=ALU.add)
            all2 = allgather(h1src[:], ag2_in, ag2_out, PARITY, 4, HW, "all2sb")

            # parity-group slice order [q, q+2, q+4, q+6]: chain (stack,dir)
            # sits at slice stack + 2*dir; gin chunk order (0,0),(0,1),(1,0),(1,1)
            mp = mseq[:, 0:1]
            ms = mseq[:, 1:2]
            gin_flat = ginT[:].rearrange('p a b -> p (a b)')
            for gi, (st, dr) in enumerate([(0, 0), (0, 1), (1, 0), (1, 1)]):
                nc.vector.tensor_copy(gin_flat[:, gi * HW:(gi + 1) * HW],
                                      all2[:, st + 2 * dr, :])

            # ---- helpers for post stages
            def ln_(x, gam, bet, out, F):
                s = sm.tile([64, 1], F32, tag="ln_s")
                nc.vector.reduce_sum(s[:], x[:, :F], axis=mybir.AxisListType.X)
                negmu = sm.tile([64, 1], F32, tag="ln_negmu")
                nc.vector.tensor_scalar_mul(negmu[:], s[:], -1.0 / F)
                xc = lnp.tile([64, 2 * C], F32, tag="ln_xc")
                nc.vector.tensor_scalar_add(xc[:, :F], x[:, :F], negmu[:])
                sq = lnp.tile([64, 2 * C], F32, tag="ln_sq")
                ssum = sm.tile([64, 1], F32, tag="ln_ssum")
                nc.vector.scalar_tensor_tensor(
                    out=sq[:, :F], in0=xc[:, :F], scalar=1.0, in1=xc[:, :F],
                    op0=ALU.mult, op1=ALU.mult, accum_out=ssum[:])
                var = sm.tile([64, 1], F32, tag="ln_var")
                nc.vector.tensor_scalar_mul(var[:], ssum[:], 1.0 / F)
                sd = sm.tile([64, 1], F32, tag="ln_sd")
                nc.scalar.activation(sd[:], var[:], AF.Sqrt, bias=eps_t[:])
                rstd = sm.tile([64, 1], F32, tag="ln_rstd")
                nc.vector.reciprocal(rstd[:], sd[:])
                nc.vector.tensor_scalar_mul(xc[:, :F], xc[:, :F], rstd[:])
                nc.vector.tensor_mul(xc[:, :F], xc[:, :F], gam[:, :F])
                nc.vector.tensor_add(out[:, :F], xc[:, :F], bet[:, :F])

            def rep_tile(d, F):
                t = repp.tile([64, 2 * C], BF16, tag="rep")
                nc.sync.dma_start(t[:, :F], d[:])
                return t

            def transpose_to(dstT, src_nat, nchunks, ident, tagsuffix=""):
                """src_nat [64, nchunks*128] -> dstT [128, nchunks, T] bf16."""
                for kc in range(nchunks):
                    tp = ps2.tile([128, 64], F32, tag="tp")
                    nc.tensor.transpose(tp[:], src_nat[:, kc * 128:(kc + 1) * 128],
                                        ident[:64, :64])
                    nc.vector.tensor_copy(dstT[:, kc, :], tp[:])

            eps_t = wp.tile([64, 1], F32)
            nc.gpsimd.memset(eps_t[:], 1e-5)
            natA = ap.tile([64, 2 * C], F32, tag="natA")
            natB = ap.tile([64, 2 * C], F32, tag="natB")

            # ================= gate stage =================
            gps = ps.tile([64, 2 * C], F32, tag="mm")
            for ns in range(2):
                gw1_h = pwp.tile([128, 2 * KC_IN, 512], BF16, tag="big")
                nc.sync.dma_start(gw1_h[:], gw1m_d[:, :, ns * 512:(ns + 1) * 512])
                for kc in range(2 * KC_IN):
                    nc.tensor.matmul(
                        gps[:, ns * 512:(ns + 1) * 512],
                        ginT[:, kc, :],
                        gw1_h[:, kc, :],
                        start=(kc == 0), stop=(kc == 2 * KC_IN - 1))
            gb1r = rep_tile(gb1r_d, C)
            nc.vector.tensor_add(natA[:, :C], gps[:, :C], gb1r[:, :C])
            glgr = rep_tile(glgr_d, C)
            glbr = rep_tile(glbr_d, C)
            ln_(natA, glgr, glbr, natB, C)
            nc.scalar.activation(natA[:, :C], natB[:, :C], AF.Gelu)

            g_hT = ap.tile([128, KC_IN, T], BF16, tag="g_hT")
            transpose_to(g_hT, natA, KC_IN, If32)

            gb2c = wp.tile([128, MC_G], F32)
            nc.sync.dma_start(gb2c[:], gb2c_d[:])
            gatesT = ap.tile([128, MC_G, T], BF16, tag="gatesT")
            for half in range(2):
                gw2_h = pwp.tile([128, KC_IN, MC_G // 2, 128], BF16, tag="big")
                nc.sync.dma_start(gw2_h[:], gw2s_d[:, :, half * 8:(half + 1) * 8, :])
                gtv = ps.tile([128, 1024], F32, tag="mm")
                for mc in range(MC_G // 2):
                    m = half * (MC_G // 2) + mc
                    for kc in range(KC_IN):
                        nc.tensor.matmul(
                            gtv[:, mc * T:(mc + 1) * T],
                            gw2_h[:, kc, mc, :],
                            g_hT[:, kc, :],
                            start=(kc == 0), stop=(kc == KC_IN - 1))
                    nc.scalar.activation(
                        gatesT[:, m, :], gtv[:, mc * T:(mc + 1) * T],
                        AF.Sigmoid, bias=gb2c[:, m:m + 1])

            biT = ap.tile([128, KC_IN, T], BF16, tag="biT")
            gflat = gatesT[:].rearrange('p a b -> p (a b)')
            ginf = ginT[:].rearrange('p a b -> p (a b)')
            bif = biT[:].rearrange('p a b -> p (a b)')
            t1 = sm.tile([128, KC_IN * T], BF16, tag="bi_t1")
            nc.vector.tensor_mul(t1[:], gflat[:, 0:512], ginf[:, 0:512])
            nc.vector.tensor_mul(bif[:], gflat[:, 512:1024], ginf[:, 512:1024])
            nc.vector.tensor_add(bif[:], bif[:], t1[:])

            # exchange biT between sequences (pad AG buffer reuse shape)
            bqc = wp.tile([128, KC_IN], F32)
            nc.sync.dma_start(bqc[:], bqc_d[:])
            bkc = wp.tile([128, KC_IN], F32)
            nc.sync.dma_start(bkc[:], bkc_d[:])

            def head_proj(dst, wsd, bc, scale, rhs):
                ws = pwp.tile([128, KC_IN, KC_IN, 128], BF16, tag="big", name="wshp")
                nc.sync.dma_start(ws[:], wsd[:])
                qv = ps.tile([128, 1024], F32, tag="mm", name="qvhp")
                for mc in range(KC_IN):
                    for kc in range(KC_IN):
                        nc.tensor.matmul(
                            qv[:, mc * T:(mc + 1) * T],
                            ws[:, kc, mc, :],
                            rhs[:, kc, :],
                            start=(kc == 0), stop=(kc == KC_IN - 1))
                    nc.vector.tensor_scalar(
                        out=dst[:, mc, :], in0=qv[:, mc * T:(mc + 1) * T],
                        scalar1=bc[:, mc:mc + 1], scalar2=scale,
                        op0=ALU.add, op1=ALU.mult)

            qT = ap.tile([128, KC_IN, T], BF16, tag="qT")
            head_proj(qT, wqs_d, bqc, 1.0 / math.sqrt(HD), biT)
            # pre-transpose bi to natural layout while the exchange runs
            bn = ap.tile([64, C], F32, tag="bn")
            for kc in range(KC_IN):
                tprb = ps2.tile([64, 128], BF16, tag="tpb", name="tprb")
                nc.tensor.transpose(tprb[:], biT[:, kc, :], Ibf[:, :])
                nc.vector.tensor_copy(bn[:, kc * 128:(kc + 1) * 128], tprb[:])

            all3 = allgather(bif[:], ag3_in, ag3_out, PAIR_SEQ, 2, 2 * HW, "all3sb")
            kvT = ap.tile([128, KC_IN, T], BF16, tag="kvT")
            kvf = kvT[:].rearrange('p a b -> p (a b)')
            nc.vector.tensor_scalar_mul(kvf[:], all3[:, 1, :], mp)
            nc.vector.scalar_tensor_tensor(
                out=kvf[:], in0=all3[:, 0, :], scalar=ms, in1=kvf[:],
                op0=ALU.mult, op1=ALU.add)

            # ================= attention =================
            kT = ap.tile([128, KC_IN, T], BF16, tag="kT")
            head_proj(kT, wks_d, bkc, 1.0, kvT)

            wvm = pwp.tile([128, KC_IN, C], BF16, tag="big")
            nc.sync.dma_start(wvm[:], wvm_d[:])
            vps = ps.tile([64, 2 * C], F32, tag="mm")
            for ns in range(2):
                for kc in range(KC_IN):
                    nc.tensor.matmul(
                        vps[:, ns * 512:(ns + 1) * 512],
                        kvT[:, kc, :],
                        wvm[:, kc, ns * 512:(ns + 1) * 512],
                        start=(kc == 0), stop=(kc == KC_IN - 1))
            bvr = rep_tile(bvr_d, C)
            v_sb = ap.tile([64, C], BF16, tag="v_sb")
            nc.vector.tensor_add(v_sb[:], vps[:, :C], bvr[:, :C])

            sps = ps2.tile([64, NH, T], F32, tag="tp")
            for h in range(NH):
                nc.tensor.matmul(sps[:, h, :], qT[:, h, :], kT[:, h, :],
                                 start=True, stop=True)
            negmax = sm.tile([64, NH], F32, tag="negmax")
            nc.vector.tensor_reduce(negmax[:], sps[:], axis=mybir.AxisListType.X,
                                    op=ALU.max, negate=True)
            attn = ap.tile([64, NH, T], F32, tag="attn")
            sumexp = sm.tile([64, NH], F32, tag="sumexp")
            for h in range(NH):
                nc.scalar.activation(attn[:, h, :], sps[:, h, :], AF.Exp,
                                     bias=negmax[:, h:h + 1],
                                     accum_out=sumexp[:, h:h + 1])
            recip = sm.tile([64, NH], F32, tag="recip")
            nc.vector.reciprocal(recip[:], sumexp[:])

            attnT = ap.tile([64, NH, T], BF16, tag="attnT")
            for h in range(NH):
                tpa = ps2.tile([64, 64], F32, tag="tp")
                nc.tensor.transpose(tpa[:], attn[:, h, :], If32[:64, :64])
                nc.vector.tensor_copy(attnT[:, h, :], tpa[:])

            aops = ps.tile([64, 2 * C], F32, tag="mm")
            for h in range(NH):
                nc.tensor.matmul(aops[:, h * HD:(h + 1) * HD],
                                 attnT[:, h, :], v_sb[:, h * HD:(h + 1) * HD],
                                 start=True, stop=True)
            ao = ap.tile([64, C], BF16, tag="ao")
            for h in range(NH):
                nc.vector.tensor_scalar_mul(
                    ao[:, h * HD:(h + 1) * HD], aops[:, h * HD:(h + 1) * HD],
                    recip[:, h:h + 1])
            aoT = ap.tile([128, KC_IN, T], BF16, tag="aoT")
            for kc in range(KC_IN):
                tpb = ps2.tile([128, 64], BF16, tag="tpb")
                nc.tensor.transpose(tpb[:], ao[:, kc * 128:(kc + 1) * 128], Ibf[:64, :64])
                nc.vector.tensor_copy(aoT[:, kc, :], tpb[:])

            wom = pwp.tile([128, KC_IN, C], BF16, tag="big")
            nc.sync.dma_start(wom[:], wom_d[:])
            paps = ps.tile([64, 2 * C], F32, tag="mm")
            for ns in range(2):
                for kc in range(KC_IN):
                    nc.tensor.matmul(
                        paps[:, ns * 512:(ns + 1) * 512],
                        aoT[:, kc, :],
                        wom[:, kc, ns * 512:(ns + 1) * 512],
                        start=(kc == 0), stop=(kc == KC_IN - 1))

            # residual: bi natural + pa + bo
            nc.vector.tensor_add(natA[:, :C], bn[:], paps[:, :C])
            bor = rep_tile(bor_d, C)
            nc.vector.tensor_add(natA[:, :C], natA[:, :C], bor[:, :C])

            lngr = rep_tile(lngr_d, C)
            lnbr = rep_tile(lnbr_d, C)
            ln_(natA, lngr, lnbr, natB, C)   # natB = po_pre

            # ================= output transform =================
            ppT = ap.tile([128, KC_IN, T], BF16, tag="ppT")
            transpose_to(ppT, natB, KC_IN, If32)

            h1ps = ps.tile([64, 2 * C], F32, tag="mm")
            for half in range(2):
                ow1_h = pwp.tile([128, KC_IN, C], BF16, tag="big")
                nc.sync.dma_start(ow1_h[:], ow1m_d[:, :, half * C:(half + 1) * C])
                for ns in range(2):
                    for kc in range(KC_IN):
                        nc.tensor.matmul(
                            h1ps[:, half * C + ns * 512: half * C + (ns + 1) * 512],
                            ppT[:, kc, :],
                            ow1_h[:, kc, ns * 512:(ns + 1) * 512],
                            start=(kc == 0), stop=(kc == KC_IN - 1))
            ob1r = rep_tile(ob1r_d, 2 * C)
            nc.vector.tensor_add(natA[:], h1ps[:], ob1r[:])
            og1r = rep_tile(og1r_d, 2 * C)
            obe1r = rep_tile(obe1r_d, 2 * C)
            ln_(natA, og1r, obe1r, natB, 2 * C)
            nc.scalar.activation(natA[:], natB[:], AF.Gelu)

            h1T = ap.tile([128, 2 * KC_IN, T], BF16, tag="h1T")
            transpose_to(h1T, natA, 2 * KC_IN, If32)

            pops = ps.tile([64, 2 * C], F32, tag="mm")
            for half in range(2):
                ow2_h = pwp.tile([128, 2 * KC_IN, 512], BF16, tag="big")
                nc.sync.dma_start(ow2_h[:], ow2m_d[:, :, half * 512:(half + 1) * 512])
                for kc in range(2 * KC_IN):
                    nc.tensor.matmul(
                        pops[:, half * 512:(half + 1) * 512],
                        h1T[:, kc, :],
                        ow2_h[:, kc, :],
                        start=(kc == 0), stop=(kc == 2 * KC_IN - 1))
            ob2r = rep_tile(ob2r_d, C)
            nc.vector.tensor_add(natA[:, :C], pops[:, :C], ob2r[:, :C])
            og2r = rep_tile(og2r_d, C)
            obe2r = rep_tile(obe2r_d, C)
            ln_(natA, og2r, obe2r, natB, C)
            nc.sync.dma_start(po_d[:], natB[:, :C])

    split_sync_waits(nc)
    return nc


def _prep_core_inputs(c, emb_pe, lstm_Wih, lstm_Whh, lstm_bih, lstm_bhh,
                      attn_w, attn_b, gate_w1, gate_b1, gate_lg, gate_lb,
                      gate_w2, gate_b2, out_w1, out_b1, out_g1, out_be1,
                      out_w2, out_b2, out_g2, out_be2, ln_g, ln_b, memo=None):
    if memo is None:
        memo = {}
    seq = c & 1
    stack = (c >> 1) & 1
    dr = (c >> 2) & 1
    asc = (stack == dr)

    x = emb_pe[seq]
    order = np.arange(T) if asc else np.arange(T)[::-1]
    xl = x[order]  # local time
    embT = _bf(xl.reshape(T, KC_IN, 128).transpose(2, 1, 0))  # [128, kc, t]

    m = {"embT": embT}
    lk = ("lstm", stack, dr)
    if lk not in memo:
        dd = {}
        for l in range(2):
            dd[f"wih{l}"] = _wtiles(np.asarray(lstm_Wih[stack, l, dr]), KC_IN, MC_G)
            dd[f"whh{l}"] = _wtiles(np.asarray(lstm_Whh[stack, l, dr]), KC_H, MC_G)
            dd[f"bias{l}"] = _f32(_pchunk(np.asarray(lstm_bih[stack, l, dr])
                                          + np.asarray(lstm_bhh[stack, l, dr])))
        memo[lk] = dd
    m.update(memo[lk])

    selw = np.zeros((16,), np.float32)
    # cols: [0]=A_loc [1]=A_sl0 [2]=A_sl1 [3]=B_loc [4]=B_sl0 [5]=B_sl1
    if dr == 0:
        selw[0] = 1.0   # group A (dir0 chain) = my own H0loc
        selw[5] = 1.0   # group B (dir1 chain) = pair slice 1 (rank c&3 + 4) rev
    else:
        selw[1] = 1.0   # group A = pair slice 0 (rank c&3) rev
        selw[3] = 1.0   # group B = my own H0loc
    m["selw"] = _f32(np.broadcast_to(selw, (128, 16)))
    mseq = np.zeros((4,), np.float32)
    mseq[seq] = 1.0
    mseq[2 if asc else 3] = 1.0
    m["mseq"] = _f32(np.broadcast_to(mseq, (128, 4)))

    q = seq
    pk = ("post", q)
    if pk in memo:
        m.update(memo[pk])
        return m
    rep = lambda v: _bf(np.broadcast_to(np.asarray(v, np.float32).reshape(1, -1),
                                        (64, np.asarray(v).shape[-1])))
    base = m
    m = {}
    m["gw1m"] = _wmoving(np.asarray(gate_w1[q]), 2 * KC_IN)
    m["gb1r"] = rep(gate_b1[q])
    m["glgr"] = rep(gate_lg[q])
    m["glbr"] = rep(gate_lb[q])
    m["gw2s"] = _wtiles(np.asarray(gate_w2[q]), KC_IN, MC_G)
    m["gb2c"] = _f32(_pchunk(np.asarray(gate_b2[q])))
    m["wqs"] = _wtiles(np.asarray(attn_w[q, 0]), KC_IN, KC_IN)
    m["wks"] = _wtiles(np.asarray(attn_w[q, 1]), KC_IN, KC_IN)
    m["bqc"] = _f32(_pchunk(np.asarray(attn_b[q, 0])))
    m["bkc"] = _f32(_pchunk(np.asarray(attn_b[q, 1])))
    m["wvm"] = _wmoving(np.asarray(attn_w[q, 2]), KC_IN)
    m["bvr"] = rep(attn_b[q, 2])
    m["wom"] = _wmoving(np.asarray(attn_w[q, 3]), KC_IN)
    m["bor"] = rep(attn_b[q, 3])
    m["ow1m"] = _wmoving(np.asarray(out_w1[q]), KC_IN)
    m["ob1r"] = rep(out_b1[q])
    m["og1r"] = rep(out_g1[q])
    m["obe1r"] = rep(out_be1[q])
    m["ow2m"] = _wmoving(np.asarray(out_w2[q]), 2 * KC_IN)
    m["ob2r"] = rep(out_b2[q])
    m["og2r"] = rep(out_g2[q])
    m["obe2r"] = rep(out_be2[q])
    m["lngr"] = rep(ln_g)
    m["lnbr"] = rep(ln_b)
    memo[pk] = m
    base.update(m)
    return base


def _get_runner():
    """Build the program once and wrap it in a persistently-jitted SPMD callable."""
    if "runner" in _CACHE:
        return _CACHE["runner"]

    import jax
    from jax.sharding import Mesh, PartitionSpec
    from jax.experimental.shard_map import shard_map
    from concourse import bass2jax

    nc = build_program()
    bass2jax.install_neuronx_cc_hook()

    in_names, out_names, out_avals, zero_outs = [], [], [], []
    for alloc in nc.m.functions[0].allocations:
        if not isinstance(alloc, mybir.MemoryLocationSet):
            continue
        name = alloc.memorylocations[0].name
        pname = nc.partition_id_tensor.name if nc.partition_id_tensor else None
        if alloc.kind == "ExternalInput":
            if name != pname:
                in_names.append(name)
        elif alloc.kind == "ExternalOutput":
            shape = tuple(alloc.tensor_shape)
            dtype = mybir.dt.np(alloc.dtype)
            out_names.append(name)
            out_avals.append(jax.core.ShapedArray(shape, dtype))
            zero_outs.append(np.zeros(shape, dtype))
    n_params = len(in_names)
    all_in = in_names + out_names
    donate = tuple(range(n_params, n_params + len(out_names)))

    def _body(*args):
        operands = list(args)
        if nc.partition_id_tensor is not None:
            operands.append(bass2jax.partition_id_tensor())
        outs = bass2jax._bass_exec_p.bind(
            *operands,
            out_avals=tuple(out_avals),
            in_names=tuple(all_in + ([nc.partition_id_tensor.name]
                                     if nc.partition_id_tensor else [])),
            out_names=tuple(out_names),
            lowering_input_output_aliases=(),
            sim_require_finite=True,
            sim_require_nnan=True,
            nc=nc,
        )
        return tuple(outs)

    devices = jax.devices()[:NC]
    mesh = Mesh(np.asarray(devices), ("core",))
    pspec = (PartitionSpec("core"),)
    sharded = jax.jit(
        shard_map(_body, mesh=mesh,
                  in_specs=pspec * (n_params + len(out_names)),
                  out_specs=pspec * len(out_names), check_rep=False),
        donate_argnums=donate, keep_unused=True)

    def run(in_maps):
        concat_in = [np.concatenate([np.asarray(in_maps[c][nm])
                                     for c in range(NC)], axis=0)
                     for nm in in_names]
        concat_zero = [np.zeros((NC * z.shape[0], *z.shape[1:]), z.dtype)
                       for z in zero_outs]
        out_arrs = sharded(*concat_in, *concat_zero)
        return [
            {nm: np.asarray(out_arrs[i]).reshape(NC, *out_avals[i].shape)[c]
             for i, nm in enumerate(out_names)}
            for c in range(NC)
        ]

    run.sharded = sharded
    run.in_names = in_names
    run.out_names = out_names
    run.out_avals = out_avals
    run.zero_outs = zero_outs
    _CACHE["runner"] = run
    return run


def kernel(prefix_emb, suffix_emb, lstm_Wih, lstm_Whh, lstm_bih, lstm_bhh,
           attn_w, attn_b, gate_w1, gate_b1, gate_lg, gate_lb, gate_w2, gate_b2,
           out_w1, out_b1, out_g1, out_be1, out_w2, out_b2, out_g2, out_be2,
           ln_g, ln_b, batch_size):
    # normalize everything to host numpy before any slicing
    (prefix_emb, suffix_emb, lstm_Wih, lstm_Whh, lstm_bih, lstm_bhh,
     attn_w, attn_b, gate_w1, gate_b1, gate_lg, gate_lb, gate_w2, gate_b2,
     out_w1, out_b1, out_g1, out_be1, out_w2, out_b2, out_g2, out_be2,
     ln_g, ln_b) = [
        np.asarray(a) for a in
        (prefix_emb, suffix_emb, lstm_Wih, lstm_Whh, lstm_bih, lstm_bhh,
         attn_w, attn_b, gate_w1, gate_b1, gate_lg, gate_lb, gate_w2, gate_b2,
         out_w1, out_b1, out_g1, out_be1, out_w2, out_b2, out_g2, out_be2,
         ln_g, ln_b)]
    run = _get_runner()

    pe = _sinusoid(T, C)
    emb_pe = (np.asarray(prefix_emb, np.float32) + pe,
              np.asarray(suffix_emb, np.float32) + pe)

    memo = {}
    in_maps = [
        _prep_core_inputs(c, emb_pe, lstm_Wih, lstm_Whh, lstm_bih, lstm_bhh,
                          attn_w, attn_b, gate_w1, gate_b1, gate_lg, gate_lb,
                          gate_w2, gate_b2, out_w1, out_b1, out_g1, out_be1,
                          out_w2, out_b2, out_g2, out_be2,
                          np.asarray(ln_g, np.float32),
                          np.asarray(ln_b, np.float32), memo=memo)
        for c in range(NC)
    ]
    results = run(in_maps)
    po = np.asarray(results[0]["po_out"], np.float32)
    so = np.asarray(results[1]["po_out"], np.float32)
    b = int(batch_size)
    po_b = np.broadcast_to(po[None], (b, T, C)).copy()
    so_b = np.broadcast_to(so[None], (b, T, C)).copy()
    return po_b, so_b


# revision 23
# speedup vs baseline: 17063.7516x; 1.0067x over previous
"""Trainium2 Bass kernel for BidirectionalAttentionalPromptEncoder.

Key algebraic fact: every batch element of the reference is IDENTICAL
(the input embeddings are broadcast over batch before any compute), so we
compute a single batch element on-device and broadcast on the host.

Distribution (8 NeuronCores, SPMD single program, data-driven per-core roles):
  core c:  seq = c&1 (0=prefix,1=suffix), stack = (c>>1)&1 (forward/backward
  LSTM stack), dir = (c>>2)&1 (direction inside the bidirectional layer).
  Each core runs one LSTM chain (seq,stack,dir) for layer 0 then layer 1,
  with AllGathers to exchange the per-direction hidden histories between
  layers.  The post-LSTM stages (gating, cross-attention, output transforms)
  are computed per-sequence; the host reads the prefix output from core 0
  and the suffix output from core 1.

All matmuls run in bf16 (f32 PSUM accumulation).  Time-reversal needed by
the bidirectional scans is handled by writing each step's hidden state at
both ascending and descending offsets (H_loc / H_rev) and selecting with
per-core 0/1 mask inputs, keeping the program SPMD-uniform.
"""

import math
import sys

sys.path.insert(0, "/opt/trn_rl_repo")

import ml_dtypes
import numpy as np

import concourse.bass as bass
import concourse.mybir as mybir
import concourse.tile as tile
from concourse.bass_utils import run_bass_kernel_spmd
from concourse.masks import make_identity

BF16 = mybir.dt.bfloat16
F32 = mybir.dt.float32
AF = mybir.ActivationFunctionType
ALU = mybir.AluOpType

C = 1024
T = 64            # prefix_length == suffix_length
H2 = 512
NH = 8
HD = C // NH      # 128
G = 4 * H2        # 2048 lstm gate dim
NC = 8            # cores
KC_IN = C // 128  # 8 input-dim chunks
KC_H = H2 // 128  # 4 hidden-dim chunks
MC_G = G // 128   # 16 gate-dim chunks
HW = KC_H * T     # 256 cols of one hidden history

_CACHE = {}


def _bf(x):
    return np.ascontiguousarray(np.asarray(x, np.float32).astype(ml_dtypes.bfloat16))


def _f32(x):
    return np.ascontiguousarray(np.asarray(x, np.float32))


def _sinusoid(t, c):
    pos = np.arange(t, dtype=np.float32)[:, None]
    div = np.exp((-math.log(10000.0) * np.arange(0, c, 2, dtype=np.float32) / c)
                 .astype(np.float32)).astype(np.float32)
    pe = np.zeros((t, c), np.float32)
    pe[:, 0::2] = np.sin(pos * div)
    pe[:, 1::2] = np.cos(pos * div)
    return pe


def _wtiles(w, kc, mc):
    """w [mc*128, kc*128] -> bf16 tiles [128, kc, mc, 128]: t[p,k,m,j] = w[m*128+j, k*128+p]."""
    wb = np.asarray(w, np.float32).astype(ml_dtypes.bfloat16)
    out = wb.reshape(mc, 128, kc, 128).transpose(3, 2, 0, 1)
    return np.ascontiguousarray(out)


def _wmoving(w, kc):
    """w [n, kc*128] -> bf16 [128, kc, n]: out[p,k,n] = w[n, k*128+p]."""
    wb = np.asarray(w, np.float32).astype(ml_dtypes.bfloat16)
    n = wb.shape[0]
    out = wb.reshape(n, kc, 128).transpose(2, 1, 0)
    return np.ascontiguousarray(out)


def _pchunk(v):
    """v [m*128] -> [128, m] per-partition chunk layout."""
    m = v.shape[0] // 128
    return np.ascontiguousarray(v.reshape(m, 128).T)


def split_sync_waits(nc):
    """Walrus NO_STRUCT instructions hold limited sem-waits; split extras onto NoOps."""
    limited = (mybir.InstDrain, mybir.InstNoOp)
    fn = nc.m.functions[0]
    for blk in fn.blocks:
        newl = []
        for inst in blk.instructions:
            si = inst.sync_info
            maxw = 1
            if si is not None and len(si.on_wait) > maxw:
                waits = list(si.on_wait)
                pre, keep = waits[:-maxw], waits[-maxw:]
                for i, w in enumerate(pre):
                    nop = mybir.InstNoOp(name=f"{inst.name}-sw{i}", ins=[], outs=[])
                    nop.engine = inst.engine
                    nop.sync_info = mybir.SyncInfo(on_wait=[w], on_update=[])
                    newl.append(nop)
                si.on_wait = keep
                inst.sync_info = si
            newl.append(inst)
        blk.instructions = newl


def build_program():
    nc = bass.Bass()

    def din(name, shape, dt=BF16):
        return nc.dram_tensor(name, shape, dt, kind="ExternalInput")

    embT_d = din("embT", [128, KC_IN, T])
    wih_d = [din(f"wih{l}", [128, KC_IN, MC_G, 128]) for l in range(2)]
    whh_d = [din(f"whh{l}", [128, KC_H, MC_G, 128]) for l in range(2)]
    bias_d = [din(f"bias{l}", [128, MC_G], F32) for l in range(2)]
    selw_d = din("selw", [128, 16], F32)
    mseq_d = din("mseq", [128, 4], F32)
    gw1m_d = din("gw1m", [128, 2 * KC_IN, C])
    gw2s_d = din("gw2s", [128, KC_IN, MC_G, 128])
    gb1r_d = din("gb1r", [64, C])
    glgr_d = din("glgr", [64, C])
    glbr_d = din("glbr", [64, C])
    gb2c_d = din("gb2c", [128, MC_G], F32)
    wqs_d = din("wqs", [128, KC_IN, KC_IN, 128])
    wks_d = din("wks", [128, KC_IN, KC_IN, 128])
    bqc_d = din("bqc", [128, KC_IN], F32)
    bkc_d = din("bkc", [128, KC_IN], F32)
    wvm_d = din("wvm", [128, KC_IN, C])
    bvr_d = din("bvr", [64, C])
    wom_d = din("wom", [128, KC_IN, C])
    bor_d = din("bor", [64, C])
    ow1m_d = din("ow1m", [128, KC_IN, 2 * C])
    ob1r_d = din("ob1r", [64, 2 * C])
    og1r_d = din("og1r", [64, 2 * C])
    obe1r_d = din("obe1r", [64, 2 * C])
    ow2m_d = din("ow2m", [128, 2 * KC_IN, C])
    ob2r_d = din("ob2r", [64, C])
    og2r_d = din("og2r", [64, C])
    obe2r_d = din("obe2r", [64, C])
    lngr_d = din("lngr", [64, C])
    lnbr_d = din("lnbr", [64, C])
    po_d = nc.dram_tensor("po_out", [64, C], F32, kind="ExternalOutput")

    with tile.TileContext(nc) as tc:
        with (
            tc.tile_pool(name="w", bufs=1) as wp,          # persistent constants/state
            tc.tile_pool(name="wih", bufs=2) as wihp,      # streamed lstm input weights
            tc.tile_pool(name="whh", bufs=2) as whhp,      # streamed lstm recurrent weights
            tc.tile_pool(name="pw", bufs=2) as pwp,        # streamed post weights (16KB tiles)
            tc.tile_pool(name="rep", bufs=3) as repp,      # streamed replicated biases
            tc.tile_pool(name="sm", bufs=2) as sm,         # small working tiles
            tc.tile_pool(name="act", bufs=1) as ap,        # activations
            tc.tile_pool(name="ln", bufs=1) as lnp,        # LN scratch
            tc.tile_pool(name="ps", bufs=1, space="PSUM") as ps,
            tc.tile_pool(name="ps2", bufs=2, space="PSUM") as ps2,
            tc.tile_pool(name="dram", bufs=1, space="DRAM") as dram,
        ):
            # ---- constants / inputs to SBUF
            If32 = wp.tile([128, 128], F32)
            make_identity(nc, If32[:])
            Ibf = wp.tile([128, 128], BF16)
            nc.vector.tensor_copy(Ibf[:], If32[:])

            embT = wp.tile([128, KC_IN, T], BF16)
            nc.sync.dma_start(embT[:], embT_d[:])
            selw = wp.tile([128, 16], F32)
            nc.sync.dma_start(selw[:], selw_d[:])
            mseq = wp.tile([128, 4], F32)
            nc.sync.dma_start(mseq[:], mseq_d[:])
            bias = [wp.tile([128, MC_G], F32, tag=f"bias{l}", name=f"bias{l}") for l in range(2)]
            for l in range(2):
                nc.sync.dma_start(bias[l][:], bias_d[l][:])

            whh = [whhp.tile([128, KC_H, MC_G, 128], BF16, tag="whh", name=f"whhl{l}") for l in range(2)]
            for l in range(2):
                nc.sync.dma_start(whh[l][:], whh_d[l][:])

            # state carried across the whole LSTM section
            H = [[wp.tile([128, KC_H, T], BF16, tag=f"H{l}{b}", name=f"H{l}{b}") for b in range(2)]
                 for l in range(2)]  # H[layer][0]=loc, [1]=rev
            xT1 = wp.tile([128, KC_IN, T], BF16)
            ginT = wp.tile([128, 2 * KC_IN, T], BF16)

            ag1_in = dram.tile([128, HW], BF16, tag="agin")
            ag1_out = dram.tile([2 * 128, HW], BF16, tag="agout")
            ag2_in = dram.tile([128, HW], BF16, tag="agin2")
            ag2_out = dram.tile([4 * 128, HW], BF16, tag="agout2")
            ag3_in = dram.tile([128, 2 * HW], BF16, tag="agin3")
            ag3_out = dram.tile([2 * 128, 2 * HW], BF16, tag="agout3")

            def x_precompute(layer, rhs):
                """X = Wih @ x + biases -> [128, gate-chunk, T] f32 sbuf."""
                xsb = wp.tile([128, MC_G, T], F32, tag="X")
                for half in range(2):
                    wih_h = wihp.tile([128, KC_IN, MC_G // 2, 128], BF16, tag="wih")
                    nc.sync.dma_start(
                        wih_h[:], wih_d[layer][:, :, half * 8:(half + 1) * 8, :])
                    xps = ps.tile([128, 1024], F32, tag="mm")
                    for mc in range(MC_G // 2):
                        m = half * (MC_G // 2) + mc
                        for kc in range(KC_IN):
                            nc.tensor.matmul(
                                xps[:, mc * T:(mc + 1) * T],
                                wih_h[:, kc, mc, :],
                                rhs[:, kc, :],
                                start=(kc == 0), stop=(kc == KC_IN - 1))
                        nc.vector.tensor_scalar_add(
                            xsb[:, m, :],
                            xps[:, mc * T:(mc + 1) * T],
                            bias[layer][:, m:m + 1])
                return xsb

            def lstm_layer(layer, X):
                c_sb = sm.tile([128, KC_H], F32, tag="c_sb")
                h_sb = sm.tile([128, KC_H], BF16, tag="h_sb")
                hloc, hrev = H[layer]
                for i in range(T):
                    z = sm.tile([128, MC_G], F32, tag="z")
                    if i == 0:
                        nc.vector.tensor_copy(z[:], X[:, :, 0])
                    else:
                        zps = ps2.tile([128, MC_G], F32, tag="tp")
                        for mc in range(MC_G):
                            for kc in range(KC_H):
                                nc.tensor.matmul(
                                    zps[:, mc:mc + 1],
                                    whh[layer][:, kc, mc, :],
                                    h_sb[:, kc:kc + 1],
                                    start=(kc == 0), stop=(kc == KC_H - 1))
                        nc.vector.tensor_add(z[:], zps[:], X[:, :, i])
                    sif = sm.tile([128, 8], F32, tag="sif")
                    nc.scalar.activation(sif[:], z[:, 0:8], AF.Sigmoid)
                    tg = sm.tile([128, 4], F32, tag="tg")
                    nc.scalar.activation(tg[:], z[:, 8:12], AF.Tanh)
                    so = sm.tile([128, 4], F32, tag="so")
                    nc.scalar.activation(so[:], z[:, 12:16], AF.Sigmoid)
                    ig = sm.tile([128, 4], F32, tag="ig")
                    nc.vector.tensor_mul(ig[:], sif[:, 0:4], tg[:])
                    if i == 0:
                        nc.vector.tensor_copy(c_sb[:], ig[:])
                    else:
                        nc.vector.tensor_mul(c_sb[:], sif[:, 4:8], c_sb[:])
                        nc.vector.tensor_add(c_sb[:], c_sb[:], ig[:])
                    tc_ = sm.tile([128, 4], F32, tag="tc_")
                    nc.scalar.activation(tc_[:], c_sb[:], AF.Tanh)
                    nc.vector.tensor_mul(h_sb[:], so[:], tc_[:])
                    # write history at ascending and descending offsets
                    nc.vector.tensor_copy(hloc[:, :, i], h_sb[:])
                    nc.vector.tensor_copy(hrev[:, :, T - 1 - i], h_sb[:])

            def allgather(inp_flat, ag_in, ag_out, groups, nsl, width, name):
                all_sb = wp.tile([128, nsl, width], BF16, tag="allg", name=name)
                nc.sync.dma_start(ag_in[:], inp_flat)
                nc.gpsimd.collective_compute(
                    "AllGather", ALU.bypass,
                    ins=[ag_in.opt()], outs=[ag_out.opt()],
                    replica_groups=groups)
                for r in range(nsl):
                    nc.sync.dma_start(all_sb[:, r, :], ag_out[r * 128:(r + 1) * 128, :])
                return all_sb

            PAIR_DIR = [[c, c + 4] for c in range(4)]      # (seq,stack) pairs across dir
            PAIR_SEQ = [[2 * c, 2 * c + 1] for c in range(4)]  # across seq
            PARITY = [[0, 2, 4, 6], [1, 3, 5, 7]]

            # ================= LSTM =================
            X0 = x_precompute(0, embT)
            lstm_layer(0, X0)

            all1 = allgather(H[0][1][:].rearrange('p a b -> p (a b)'),
                             ag1_in, ag1_out, PAIR_DIR, 2, HW, "all1sb")

            # xT1 group A (chunks 0-3) = dir0-chain's history in my local time,
            # group B = dir1-chain's.  Sources: own H0loc, or pair slice 0/1
            # (rev of ranks c&3 / (c&3)+4); 0/1 weights from selw cols:
            #   [0]=A_loc [1]=A_sl0 [2]=A_sl1 [3]=B_loc [4]=B_sl0 [5]=B_sl1
            h0loc = H[0][0][:].rearrange('p a b -> p (a b)')
            xf = xT1[:].rearrange('p a b -> p (a b)')
            for g in range(2):
                dst = xf[:, g * HW:(g + 1) * HW]
                nc.vector.tensor_scalar_mul(dst, h0loc[:], selw[:, 3 * g:3 * g + 1])
                for sl in range(2):
                    nc.vector.scalar_tensor_tensor(
                        out=dst, in0=all1[:, sl, :],
                        scalar=selw[:, 3 * g + 1 + sl:3 * g + 2 + sl], in1=dst,
                        op0=ALU.mult, op1=ALU.add)

            X1 = x_precompute(1, xT1)
            lstm_layer(1, X1)

            # each core sends its chain's SOURCE-time history: loc if asc else rev
            h1src = sm.tile([128, HW], BF16, tag="h1src")
            nc.vector.tensor_scalar_mul(
                h1src[:], H[1][0][:].rearrange('p a b -> p (a b)'), mseq[:, 2:3])
            nc.vector.scalar_tensor_tensor(
                out=h1src[:], in0=H[1][1][:].rearrange('p a b -> p (a b)'),
                scalar=mseq[:, 3:4], in1=h1src[:], op0=ALU.mult, op1=ALU.add)
            all2 = allgather(h1src[:], ag2_in, ag2_out, PARITY, 4, HW, "all2sb")

            # parity-group slice order [q, q+2, q+4, q+6]: chain (stack,dir)
            # sits at slice stack + 2*dir; gin chunk order (0,0),(0,1),(1,0),(1,1)
            mp = mseq[:, 0:1]
            ms = mseq[:, 1:2]
            gin_flat = ginT[:].rearrange('p a b -> p (a b)')
            for gi, (st, dr) in enumerate([(0, 0), (0, 1), (1, 0), (1, 1)]):
                nc.vector.tensor_copy(gin_flat[:, gi * HW:(gi + 1) * HW],
                                      all2[:, st + 2 * dr, :])

            # ---- helpers for post stages
            def ln_(x, gam, bet, out, F):
                # var = E[x^2] - mu^2: both reductions read x directly (no
                # centering pass, shorter dependency chain)
                s = sm.tile([64, 1], F32, tag="ln_s")
                nc.vector.reduce_sum(s[:], x[:, :F], axis=mybir.AxisListType.X)
                sq = lnp.tile([64, 2 * C], F32, tag="ln_sq")
                ssum = sm.tile([64, 1], F32, tag="ln_ssum")
                nc.vector.scalar_tensor_tensor(
                    out=sq[:, :F], in0=x[:, :F], scalar=1.0, in1=x[:, :F],
                    op0=ALU.mult, op1=ALU.mult, accum_out=ssum[:])
                negmu = sm.tile([64, 1], F32, tag="ln_negmu")
                nc.vector.tensor_scalar_mul(negmu[:], s[:], -1.0 / F)
                mu2 = sm.tile([64, 1], F32, tag="ln_mu2")
                nc.vector.tensor_mul(mu2[:], negmu[:], negmu[:])
                var = sm.tile([64, 1], F32, tag="ln_var")
                nc.vector.tensor_scalar_mul(var[:], ssum[:], 1.0 / F)
                nc.vector.tensor_sub(var[:], var[:], mu2[:])
                sd = sm.tile([64, 1], F32, tag="ln_sd")
                nc.scalar.activation(sd[:], var[:], AF.Sqrt, bias=eps_t[:])
                rstd = sm.tile([64, 1], F32, tag="ln_rstd")
                nc.vector.reciprocal(rstd[:], sd[:])
                xc = lnp.tile([64, 2 * C], F32, tag="ln_xc")
                nc.vector.tensor_scalar(
                    out=xc[:, :F], in0=x[:, :F], scalar1=negmu[:], scalar2=rstd[:],
                    op0=ALU.add, op1=ALU.mult)
                nc.vector.tensor_mul(xc[:, :F], xc[:, :F], gam[:, :F])
                nc.vector.tensor_add(out[:, :F], xc[:, :F], bet[:, :F])

            def rep_tile(d, F):
                t = repp.tile([64, 2 * C], BF16, tag="rep")
                nc.sync.dma_start(t[:, :F], d[:])
                return t

            def transpose_to(dstT, src_nat, nchunks, ident, tagsuffix=""):
                """src_nat [64, nchunks*128] -> dstT [128, nchunks, T] bf16."""
                for kc in range(nchunks):
                    tp = ps2.tile([128, 64], F32, tag="tp")
                    nc.tensor.transpose(tp[:], src_nat[:, kc * 128:(kc + 1) * 128],
                                        ident[:64, :64])
                    nc.vector.tensor_copy(dstT[:, kc, :], tp[:])

            eps_t = wp.tile([64, 1], F32)
            nc.gpsimd.memset(eps_t[:], 1e-5)
            natA = ap.tile([64, 2 * C], F32, tag="natA")
            natB = ap.tile([64, 2 * C], F32, tag="natB")

            # ================= gate stage =================
            gps = ps.tile([64, 2 * C], F32, tag="mm")
            for ns in range(2):
                gw1_h = pwp.tile([128, 2 * KC_IN, 512], BF16, tag="big")
                nc.sync.dma_start(gw1_h[:], gw1m_d[:, :, ns * 512:(ns + 1) * 512])
                for kc in range(2 * KC_IN):
                    nc.tensor.matmul(
                        gps[:, ns * 512:(ns + 1) * 512],
                        ginT[:, kc, :],
                        gw1_h[:, kc, :],
                        start=(kc == 0), stop=(kc == 2 * KC_IN - 1))
            gb1r = rep_tile(gb1r_d, C)
            nc.vector.tensor_add(natA[:, :C], gps[:, :C], gb1r[:, :C])
            glgr = rep_tile(glgr_d, C)
            glbr = rep_tile(glbr_d, C)
            ln_(natA, glgr, glbr, natB, C)
            nc.scalar.activation(natA[:, :C], natB[:, :C], AF.Gelu)

            g_hT = ap.tile([128, KC_IN, T], BF16, tag="g_hT")
            transpose_to(g_hT, natA, KC_IN, If32)

            gb2c = wp.tile([128, MC_G], F32)
            nc.sync.dma_start(gb2c[:], gb2c_d[:])
            gatesT = ap.tile([128, MC_G, T], BF16, tag="gatesT")
            for half in range(2):
                gw2_h = pwp.tile([128, KC_IN, MC_G // 2, 128], BF16, tag="big")
                nc.sync.dma_start(gw2_h[:], gw2s_d[:, :, half * 8:(half + 1) * 8, :])
                gtv = ps.tile([128, 1024], F32, tag="mm")
                for mc in range(MC_G // 2):
                    m = half * (MC_G // 2) + mc
                    for kc in range(KC_IN):
                        nc.tensor.matmul(
                            gtv[:, mc * T:(mc + 1) * T],
                            gw2_h[:, kc, mc, :],
                            g_hT[:, kc, :],
                            start=(kc == 0), stop=(kc == KC_IN - 1))
                    nc.scalar.activation(
                        gatesT[:, m, :], gtv[:, mc * T:(mc + 1) * T],
                        AF.Sigmoid, bias=gb2c[:, m:m + 1])

            biT = ap.tile([128, KC_IN, T], BF16, tag="biT")
            gflat = gatesT[:].rearrange('p a b -> p (a b)')
            ginf = ginT[:].rearrange('p a b -> p (a b)')
            bif = biT[:].rearrange('p a b -> p (a b)')
            t1 = sm.tile([128, KC_IN * T], BF16, tag="bi_t1")
            nc.vector.tensor_mul(t1[:], gflat[:, 0:512], ginf[:, 0:512])
            nc.vector.tensor_mul(bif[:], gflat[:, 512:1024], ginf[:, 512:1024])
            nc.vector.tensor_add(bif[:], bif[:], t1[:])

            # exchange biT between sequences (pad AG buffer reuse shape)
            bqc = wp.tile([128, KC_IN], F32)
            nc.sync.dma_start(bqc[:], bqc_d[:])
            bkc = wp.tile([128, KC_IN], F32)
            nc.sync.dma_start(bkc[:], bkc_d[:])

            def head_proj(dst, wsd, bc, scale, rhs):
                ws = pwp.tile([128, KC_IN, KC_IN, 128], BF16, tag="big", name="wshp")
                nc.sync.dma_start(ws[:], wsd[:])
                qv = ps.tile([128, 1024], F32, tag="mm", name="qvhp")
                for mc in range(KC_IN):
                    for kc in range(KC_IN):
                        nc.tensor.matmul(
                            qv[:, mc * T:(mc + 1) * T],
                            ws[:, kc, mc, :],
                            rhs[:, kc, :],
                            start=(kc == 0), stop=(kc == KC_IN - 1))
                    nc.vector.tensor_scalar(
                        out=dst[:, mc, :], in0=qv[:, mc * T:(mc + 1) * T],
                        scalar1=bc[:, mc:mc + 1], scalar2=scale,
                        op0=ALU.add, op1=ALU.mult)

            qT = ap.tile([128, KC_IN, T], BF16, tag="qT")
            head_proj(qT, wqs_d, bqc, 1.0 / math.sqrt(HD), biT)
            # pre-transpose bi to natural layout while the exchange runs
            bn = ap.tile([64, C], F32, tag="bn")
            for kc in range(KC_IN):
                tprb = ps2.tile([64, 128], BF16, tag="tpb", name="tprb")
                nc.tensor.transpose(tprb[:], biT[:, kc, :], Ibf[:, :])
                nc.vector.tensor_copy(bn[:, kc * 128:(kc + 1) * 128], tprb[:])

            all3 = allgather(bif[:], ag3_in, ag3_out, PAIR_SEQ, 2, 2 * HW, "all3sb")
            kvT = ap.tile([128, KC_IN, T], BF16, tag="kvT")
            kvf = kvT[:].rearrange('p a b -> p (a b)')
            nc.vector.tensor_scalar_mul(kvf[:], all3[:, 1, :], mp)
            nc.vector.scalar_tensor_tensor(
                out=kvf[:], in0=all3[:, 0, :], scalar=ms, in1=kvf[:],
                op0=ALU.mult, op1=ALU.add)

            # ================= attention =================
            kT = ap.tile([128, KC_IN, T], BF16, tag="kT")
            head_proj(kT, wks_d, bkc, 1.0, kvT)

            wvm = pwp.tile([128, KC_IN, C], BF16, tag="big")
            nc.sync.dma_start(wvm[:], wvm_d[:])
            vps = ps.tile([64, 2 * C], F32, tag="mm")
            for ns in range(2):
                for kc in range(KC_IN):
                    nc.tensor.matmul(
                        vps[:, ns * 512:(ns + 1) * 512],
                        kvT[:, kc, :],
                        wvm[:, kc, ns * 512:(ns + 1) * 512],
                        start=(kc == 0), stop=(kc == KC_IN - 1))
            bvr = rep_tile(bvr_d, C)
            v_sb = ap.tile([64, C], BF16, tag="v_sb")
            nc.vector.tensor_add(v_sb[:], vps[:, :C], bvr[:, :C])

            sps = ps2.tile([64, NH, T], F32, tag="tp")
            for h in range(NH):
                nc.tensor.matmul(sps[:, h, :], qT[:, h, :], kT[:, h, :],
                                 start=True, stop=True)
            negmax = sm.tile([64, NH], F32, tag="negmax")
            nc.vector.tensor_reduce(negmax[:], sps[:], axis=mybir.AxisListType.X,
                                    op=ALU.max, negate=True)
            attn = ap.tile([64, NH, T], F32, tag="attn")
            sumexp = sm.tile([64, NH], F32, tag="sumexp")
            for h in range(NH):
                nc.scalar.activation(attn[:, h, :], sps[:, h, :], AF.Exp,
                                     bias=negmax[:, h:h + 1],
                                     accum_out=sumexp[:, h:h + 1])
            recip = sm.tile([64, NH], F32, tag="recip")
            nc.vector.reciprocal(recip[:], sumexp[:])

            attnT = ap.tile([64, NH, T], BF16, tag="attnT")
            for h in range(NH):
                tpa = ps2.tile([64, 64], F32, tag="tp")
                nc.tensor.transpose(tpa[:], attn[:, h, :], If32[:64, :64])
                nc.vector.tensor_copy(attnT[:, h, :], tpa[:])

            aops = ps.tile([64, 2 * C], F32, tag="mm")
            for h in range(NH):
                nc.tensor.matmul(aops[:, h * HD:(h + 1) * HD],
                                 attnT[:, h, :], v_sb[:, h * HD:(h + 1) * HD],
                                 start=True, stop=True)
            ao = ap.tile([64, C], BF16, tag="ao")
            for h in range(NH):
                nc.vector.tensor_scalar_mul(
                    ao[:, h * HD:(h + 1) * HD], aops[:, h * HD:(h + 1) * HD],
                    recip[:, h:h + 1])
            aoT = ap.tile([128, KC_IN, T], BF16, tag="aoT")
            for kc in range(KC_IN):
                tpb = ps2.tile([128, 64], BF16, tag="tpb")
                nc.tensor.transpose(tpb[:], ao[:, kc * 128:(kc + 1) * 128], Ibf[:64, :64])
                nc.vector.tensor_copy(aoT[:, kc, :], tpb[:])

            wom = pwp.tile([128, KC_IN, C], BF16, tag="big")
            nc.sync.dma_start(wom[:], wom_d[:])
            paps = ps.tile([64, 2 * C], F32, tag="mm")
            for ns in range(2):
                for kc in range(KC_IN):
                    nc.tensor.matmul(
                        paps[:, ns * 512:(ns + 1) * 512],
                        aoT[:, kc, :],
                        wom[:, kc, ns * 512:(ns + 1) * 512],
                        start=(kc == 0), stop=(kc == KC_IN - 1))

            # residual: bi natural + pa + bo
            nc.vector.tensor_add(natA[:, :C], bn[:], paps[:, :C])
            bor = rep_tile(bor_d, C)
            nc.vector.tensor_add(natA[:, :C], natA[:, :C], bor[:, :C])

            lngr = rep_tile(lngr_d, C)
            lnbr = rep_tile(lnbr_d, C)
            ln_(natA, lngr, lnbr, natB, C)   # natB = po_pre

            # ================= output transform =================
            ppT = ap.tile([128, KC_IN, T], BF16, tag="ppT")
            transpose_to(ppT, natB, KC_IN, If32)

            h1ps = ps.tile([64, 2 * C], F32, tag="mm")
            for half in range(2):
                ow1_h = pwp.tile([128, KC_IN, C], BF16, tag="big")
                nc.sync.dma_start(ow1_h[:], ow1m_d[:, :, half * C:(half + 1) * C])
                for ns in range(2):
                    for kc in range(KC_IN):
                        nc.tensor.matmul(
                            h1ps[:, half * C + ns * 512: half * C + (ns + 1) * 512],
                            ppT[:, kc, :],
                            ow1_h[:, kc, ns * 512:(ns + 1) * 512],
                            start=(kc == 0), stop=(kc == KC_IN - 1))
            ob1r = rep_tile(ob1r_d, 2 * C)
            nc.vector.tensor_add(natA[:], h1ps[:], ob1r[:])
            og1r = rep_tile(og1r_d, 2 * C)
            obe1r = rep_tile(obe1r_d, 2 * C)
            ln_(natA, og1r, obe1r, natB, 2 * C)
            nc.scalar.activation(natA[:], natB[:], AF.Gelu)

            h1T = ap.tile([128, 2 * KC_IN, T], BF16, tag="h1T")
            transpose_to(h1T, natA, 2 * KC_IN, If32)

            pops = ps.tile([64, 2 * C], F32, tag="mm")
            for half in range(2):
                ow2_h = pwp.tile([128, 2 * KC_IN, 512], BF16, tag="big")
                nc.sync.dma_start(ow2_h[:], ow2m_d[:, :, half * 512:(half + 1) * 512])
                for kc in range(2 * KC_IN):
                    nc.tensor.matmul(
                        pops[:, half * 512:(half + 1) * 512],
                        h1T[:, kc, :],
                        ow2_h[:, kc, :],
                        start=(kc == 0), stop=(kc == 2 * KC_IN - 1))
            ob2r = rep_tile(ob2r_d, C)
            nc.vector.tensor_add(natA[:, :C], pops[:, :C], ob2r[:, :C])
            og2r = rep_tile(og2r_d, C)
            obe2r = rep_tile(obe2r_d, C)
            ln_(natA, og2r, obe2r, natB, C)
            nc.sync.dma_start(po_d[:], natB[:, :C])

    split_sync_waits(nc)
    return nc


def _prep_core_inputs(c, emb_pe, lstm_Wih, lstm_Whh, lstm_bih, lstm_bhh,
                      attn_w, attn_b, gate_w1, gate_b1, gate_lg, gate_lb,
                      gate_w2, gate_b2, out_w1, out_b1, out_g1, out_be1,
                      out_w2, out_b2, out_g2, out_be2, ln_g, ln_b, memo=None):
    if memo is None:
        memo = {}
    seq = c & 1
    stack = (c >> 1) & 1
    dr = (c >> 2) & 1
    asc = (stack == dr)

    x = emb_pe[seq]
    order = np.arange(T) if asc else np.arange(T)[::-1]
    xl = x[order]  # local time
    embT = _bf(xl.reshape(T, KC_IN, 128).transpose(2, 1, 0))  # [128, kc, t]

    m = {"embT": embT}
    lk = ("lstm", stack, dr)
    if lk not in memo:
        dd = {}
        for l in range(2):
            dd[f"wih{l}"] = _wtiles(np.asarray(lstm_Wih[stack, l, dr]), KC_IN, MC_G)
            dd[f"whh{l}"] = _wtiles(np.asarray(lstm_Whh[stack, l, dr]), KC_H, MC_G)
            dd[f"bias{l}"] = _f32(_pchunk(np.asarray(lstm_bih[stack, l, dr])
                                          + np.asarray(lstm_bhh[stack, l, dr])))
        memo[lk] = dd
    m.update(memo[lk])

    selw = np.zeros((16,), np.float32)
    # cols: [0]=A_loc [1]=A_sl0 [2]=A_sl1 [3]=B_loc [4]=B_sl0 [5]=B_sl1
    if dr == 0:
        selw[0] = 1.0   # group A (dir0 chain) = my own H0loc
        selw[5] = 1.0   # group B (dir1 chain) = pair slice 1 (rank c&3 + 4) rev
    else:
        selw[1] = 1.0   # group A = pair slice 0 (rank c&3) rev
        selw[3] = 1.0   # group B = my own H0loc
    m["selw"] = _f32(np.broadcast_to(selw, (128, 16)))
    mseq = np.zeros((4,), np.float32)
    mseq[seq] = 1.0
    mseq[2 if asc else 3] = 1.0
    m["mseq"] = _f32(np.broadcast_to(mseq, (128, 4)))

    q = seq
    pk = ("post", q)
    if pk in memo:
        m.update(memo[pk])
        return m
    rep = lambda v: _bf(np.broadcast_to(np.asarray(v, np.float32).reshape(1, -1),
                                        (64, np.asarray(v).shape[-1])))
    base = m
    m = {}
    m["gw1m"] = _wmoving(np.asarray(gate_w1[q]), 2 * KC_IN)
    m["gb1r"] = rep(gate_b1[q])
    m["glgr"] = rep(gate_lg[q])
    m["glbr"] = rep(gate_lb[q])
    m["gw2s"] = _wtiles(np.asarray(gate_w2[q]), KC_IN, MC_G)
    m["gb2c"] = _f32(_pchunk(np.asarray(gate_b2[q])))
    m["wqs"] = _wtiles(np.asarray(attn_w[q, 0]), KC_IN, KC_IN)
    m["wks"] = _wtiles(np.asarray(attn_w[q, 1]), KC_IN, KC_IN)
    m["bqc"] = _f32(_pchunk(np.asarray(attn_b[q, 0])))
    m["bkc"] = _f32(_pchunk(np.asarray(attn_b[q, 1])))
    m["wvm"] = _wmoving(np.asarray(attn_w[q, 2]), KC_IN)
    m["bvr"] = rep(attn_b[q, 2])
    m["wom"] = _wmoving(np.asarray(attn_w[q, 3]), KC_IN)
    m["bor"] = rep(attn_b[q, 3])
    m["ow1m"] = _wmoving(np.asarray(out_w1[q]), KC_IN)
    m["ob1r"] = rep(out_b1[q])
    m["og1r"] = rep(out_g1[q])
    m["obe1r"] = rep(out_be1[q])
    m["ow2m"] = _wmoving(np.asarray(out_w2[q]), 2 * KC_IN)
    m["ob2r"] = rep(out_b2[q])
    m["og2r"] = rep(out_g2[q])
    m["obe2r"] = rep(out_be2[q])
    m["lngr"] = rep(ln_g)
    m["lnbr"] = rep(ln_b)
    memo[pk] = m
    base.update(m)
    return base


def _get_runner():
    """Build the program once and wrap it in a persistently-jitted SPMD callable."""
    if "runner" in _CACHE:
        return _CACHE["runner"]

    import jax
    from jax.sharding import Mesh, PartitionSpec
    from jax.experimental.shard_map import shard_map
    from concourse import bass2jax

    nc = build_program()
    bass2jax.install_neuronx_cc_hook()

    in_names, out_names, out_avals, zero_outs = [], [], [], []
    for alloc in nc.m.functions[0].allocations:
        if not isinstance(alloc, mybir.MemoryLocationSet):
            continue
        name = alloc.memorylocations[0].name
        pname = nc.partition_id_tensor.name if nc.partition_id_tensor else None
        if alloc.kind == "ExternalInput":
            if name != pname:
                in_names.append(name)
        elif alloc.kind == "ExternalOutput":
            shape = tuple(alloc.tensor_shape)
            dtype = mybir.dt.np(alloc.dtype)
            out_names.append(name)
            out_avals.append(jax.core.ShapedArray(shape, dtype))
            zero_outs.append(np.zeros(shape, dtype))
    n_params = len(in_names)
    all_in = in_names + out_names
    donate = tuple(range(n_params, n_params + len(out_names)))

    def _body(*args):
        operands = list(args)
        if nc.partition_id_tensor is not None:
            operands.append(bass2jax.partition_id_tensor())
        outs = bass2jax._bass_exec_p.bind(
            *operands,
            out_avals=tuple(out_avals),
            in_names=tuple(all_in + ([nc.partition_id_tensor.name]
                                     if nc.partition_id_tensor else [])),
            out_names=tuple(out_names),
            lowering_input_output_aliases=(),
            sim_require_finite=True,
            sim_require_nnan=True,
            nc=nc,
        )
        return tuple(outs)

    devices = jax.devices()[:NC]
    mesh = Mesh(np.asarray(devices), ("core",))
    pspec = (PartitionSpec("core"),)
    sharded = jax.jit(
        shard_map(_body, mesh=mesh,
                  in_specs=pspec * (n_params + len(out_names)),
                  out_specs=pspec * len(out_names), check_rep=False),
        donate_argnums=donate, keep_unused=True)

    def run(in_maps):
        concat_in = [np.concatenate([np.asarray(in_maps[c][nm])
                                     for c in range(NC)], axis=0)
                     for nm in in_names]
        concat_zero = [np.zeros((NC * z.shape[0], *z.shape[1:]), z.dtype)
                       for z in zero_outs]
        out_arrs = sharded(*concat_in, *concat_zero)
        return [
            {nm: np.asarray(out_arrs[i]).reshape(NC, *out_avals[i].shape)[c]
             for i, nm in enumerate(out_names)}
            for c in range(NC)
        ]

    run.sharded = sharded
    run.in_names = in_names
    run.out_names = out_names
    run.out_avals = out_avals
    run.zero_outs = zero_outs
    _CACHE["runner"] = run
    return run


def kernel(prefix_emb, suffix_emb, lstm_Wih, lstm_Whh, lstm_bih, lstm_bhh,
           attn_w, attn_b, gate_w1, gate_b1, gate_lg, gate_lb, gate_w2, gate_b2,
           out_w1, out_b1, out_g1, out_be1, out_w2, out_b2, out_g2, out_be2,
           ln_g, ln_b, batch_size):
    # normalize everything to host numpy before any slicing
    (prefix_emb, suffix_emb, lstm_Wih, lstm_Whh, lstm_bih, lstm_bhh,
     attn_w, attn_b, gate_w1, gate_b1, gate_lg, gate_lb, gate_w2, gate_b2,
     out_w1, out_b1, out_g1, out_be1, out_w2, out_b2, out_g2, out_be2,
     ln_g, ln_b) = [
        np.asarray(a) for a in
        (prefix_emb, suffix_emb, lstm_Wih, lstm_Whh, lstm_bih, lstm_bhh,
         attn_w, attn_b, gate_w1, gate_b1, gate_lg, gate_lb, gate_w2, gate_b2,
         out_w1, out_b1, out_g1, out_be1, out_w2, out_b2, out_g2, out_be2,
         ln_g, ln_b)]
    run = _get_runner()

    pe = _sinusoid(T, C)
    emb_pe = (np.asarray(prefix_emb, np.float32) + pe,
              np.asarray(suffix_emb, np.float32) + pe)

    memo = {}
    in_maps = [
        _prep_core_inputs(c, emb_pe, lstm_Wih, lstm_Whh, lstm_bih, lstm_bhh,
                          attn_w, attn_b, gate_w1, gate_b1, gate_lg, gate_lb,
                          gate_w2, gate_b2, out_w1, out_b1, out_g1, out_be1,
                          out_w2, out_b2, out_g2, out_be2,
                          np.asarray(ln_g, np.float32),
                          np.asarray(ln_b, np.float32), memo=memo)
        for c in range(NC)
    ]
    results = run(in_maps)
    po = np.asarray(results[0]["po_out"], np.float32)
    so = np.asarray(results[1]["po_out"], np.float32)
    b = int(batch_size)
    po_b = np.broadcast_to(po[None], (b, T, C)).copy()
    so_b = np.broadcast_to(so[None], (b, T, C)).copy()
    return po_b, so_b
